# revision 4
# baseline (speedup 1.0000x reference)
"""MoE all-to-all dispatch + combine (nn_EpAll2AllFusedOp) on 8 trn2 NeuronCores.

Semantics (matching the jax reference):
  flat_expert = topk_idx.reshape(T*K)
  sort_idx    = stable argsort of flat_expert
  dispatched  = x[sort_idx // K]                      # [T*K, H], expert-contiguous
  combined[t] = x[t] * sum_k topk_weights[t, k]      # gather-back of the K copies
  tokens_per_expert = histogram(flat_expert, 64)     # int32

Sharding: the dispatched buffer (T*K = 32768 rows, the 512 MB payload) is
split into 8 equal contiguous row slices - one per core (the expert-parallel
split of the sorted/A2A'd buffer, load-balanced by slot rather than raw
expert id). Each core produces its 4096-row slice of `dispatched` plus its
512-token slice of `combined`. The sort itself is O(T*K) integer metadata
computed on host; all tensor traffic runs on-device.

Two device strategies:
  simple: indirect-gather x[src_token[slot]] per 128-slot tile, direct store.
          ~151 MB/core of DMA traffic.
  dedup:  gather each DISTINCT source row once (~2700 of 4096 slots are
          unique), then indirect-scatter it to all its output slots,
          OOB-masked via bounds_check. ~128 MB/core -> ~15% less traffic.
"""

import numpy as np

import concourse.bass as bass
import concourse.mybir as mybir
import concourse.tile as tile
from concourse import bacc
from concourse.bass_utils import run_bass_kernel_spmd

T = 4096          # tokens
H = 4096          # hidden
K = 8             # topk
E = 64            # experts
NCORES = 8
RPC = T * K // NCORES       # dispatched rows per core = 4096
NTILES = RPC // 128         # gather tiles per core = 32
CTOK = T // NCORES          # combine tokens per core = 512
CTILES = CTOK // 128        # combine tiles per core = 4

FP32 = mybir.dt.float32
I32 = mybir.dt.int32
SENT = T                    # OOB sentinel row index (skipped via bounds_check;
                            # small value so index*row_stride never overflows i32)


def _emit_combine(nc, cpool, meta, w, xc, comb):
    w_sb = meta.tile([128, CTILES * K], FP32, name="w_sb")
    nc.sync.dma_start(out=w_sb[:], in_=w[:])
    ws = [meta.tile([128, 1], FP32, name=f"ws{j}", tag=f"ws{j}")
          for j in range(CTILES)]
    for j in range(CTILES):
        nc.vector.reduce_sum(
            out=ws[j][:], in_=w_sb[:, j * K:(j + 1) * K],
            axis=mybir.AxisListType.X,
        )
    for j in range(CTILES):
        xt = cpool.tile([128, H], FP32, name="xt", tag="c")
        nc.scalar.dma_start(out=xt[:], in_=xc[j * 128:(j + 1) * 128, :])
        nc.vector.tensor_scalar_mul(out=xt[:], in0=xt[:], scalar1=ws[j][:])
        nc.sync.dma_start(out=comb[j * 128:(j + 1) * 128, :], in_=xt[:])


def build_module(gather_bufs: int = 6, comb_bufs: int = 3):
    """Simple variant: one indirect gather + one direct store per 128 slots."""
    nc = bacc.Bacc("TRN2", num_devices=NCORES)
    x = nc.dram_tensor("x", [T, H], FP32, kind="ExternalInput")
    idx = nc.dram_tensor("idx", [128, NTILES], I32, kind="ExternalInput")
    w = nc.dram_tensor("w", [128, CTILES * K], FP32, kind="ExternalInput")
    xc = nc.dram_tensor("xc", [CTOK, H], FP32, kind="ExternalInput")
    disp = nc.dram_tensor("disp", [RPC, H], FP32, kind="ExternalOutput")
    comb = nc.dram_tensor("comb", [CTOK, H], FP32, kind="ExternalOutput")

    with tile.TileContext(nc) as tc:
        with (
            tc.tile_pool(name="meta", bufs=1) as meta,
            tc.tile_pool(name="gpool", bufs=gather_bufs) as gpool,
            tc.tile_pool(name="cpool", bufs=comb_bufs) as cpool,
        ):
            idx_sb = meta.tile([128, NTILES], I32, name="idx_sb")
            nc.sync.dma_start(out=idx_sb[:], in_=idx[:])
            for i in range(NTILES):
                g = gpool.tile([128, H], FP32, name="g", tag="g")
                nc.gpsimd.indirect_dma_start(
                    out=g[:],
                    out_offset=None,
                    in_=x[:],
                    in_offset=bass.IndirectOffsetOnAxis(
                        ap=idx_sb[:, i:i + 1], axis=0),
                )
                nc.sync.dma_start(out=disp[i * 128:(i + 1) * 128, :], in_=g[:])
            _emit_combine(nc, cpool, meta, w, xc, comb)
    nc.compile()
    return nc


def build_module_dedup(n_chunks: int, passes: tuple,
                       gather_bufs: int = 6, comb_bufs: int = 3):
    """Dedup variant: gather distinct rows once, indirect-scatter to all slots.

    n_chunks: number of 128-row unique-token chunks (same on all cores,
    OOB-padded). passes[j]: scatter passes for chunk j (max multiplicity over
    cores; masked slots move no bytes).
    """
    sp = sum(passes)
    nc = bacc.Bacc("TRN2", num_devices=NCORES)
    x = nc.dram_tensor("x", [T, H], FP32, kind="ExternalInput")
    idxg = nc.dram_tensor("idxg", [128, n_chunks], I32, kind="ExternalInput")
    idxs = nc.dram_tensor("idxs", [128, sp], I32, kind="ExternalInput")
    w = nc.dram_tensor("w", [128, CTILES * K], FP32, kind="ExternalInput")
    xc = nc.dram_tensor("xc", [CTOK, H], FP32, kind="ExternalInput")
    disp = nc.dram_tensor("disp", [RPC, H], FP32, kind="ExternalOutput")
    comb = nc.dram_tensor("comb", [CTOK, H], FP32, kind="ExternalOutput")

    with tile.TileContext(nc) as tc:
        with (
            tc.tile_pool(name="meta", bufs=1) as meta,
            tc.tile_pool(name="gpool", bufs=gather_bufs) as gpool,
            tc.tile_pool(name="cpool", bufs=comb_bufs) as cpool,
        ):
            idxg_sb = meta.tile([128, n_chunks], I32, name="idxg_sb")
            nc.sync.dma_start(out=idxg_sb[:], in_=idxg[:])
            idxs_sb = meta.tile([128, sp], I32, name="idxs_sb")
            nc.sync.dma_start(out=idxs_sb[:], in_=idxs[:])
            col = 0
            for j in range(n_chunks):
                g = gpool.tile([128, H], FP32, name="g", tag="g")
                nc.gpsimd.indirect_dma_start(
                    out=g[:],
                    out_offset=None,
                    in_=x[:],
                    in_offset=bass.IndirectOffsetOnAxis(
                        ap=idxg_sb[:, j:j + 1], axis=0),
                    bounds_check=T - 1,
                    oob_is_err=False,
                )
                for _ in range(passes[j]):
                    nc.gpsimd.indirect_dma_start(
                        out=disp[:],
                        out_offset=bass.IndirectOffsetOnAxis(
                            ap=idxs_sb[:, col:col + 1], axis=0),
                        in_=g[:],
                        in_offset=None,
                        bounds_check=RPC - 1,
                        oob_is_err=False,
                    )
                    col += 1
            _emit_combine(nc, cpool, meta, w, xc, comb)
    nc.compile()
    return nc


def _routing(topk_idx):
    flat = np.ascontiguousarray(topk_idx, dtype=np.int32).reshape(-1)
    sort_idx = np.argsort(flat, kind="stable")
    src_tok = (sort_idx // K).astype(np.int32)
    tokens_per_expert = np.bincount(flat, minlength=E).astype(topk_idx.dtype)
    return src_tok, tokens_per_expert


def _common_inputs(x, topk_weights, c):
    wc = np.ascontiguousarray(topk_weights, dtype=np.float32)[
        c * CTOK:(c + 1) * CTOK]
    w_arr = np.ascontiguousarray(
        wc.reshape(CTILES, 128, K).transpose(1, 0, 2).reshape(128, CTILES * K))
    return {"w": w_arr, "xc": np.ascontiguousarray(x[c * CTOK:(c + 1) * CTOK])}


def make_in_maps(x, topk_idx, topk_weights):
    """Simple variant inputs."""
    x = np.ascontiguousarray(x, dtype=np.float32)
    src_tok, tokens_per_expert = _routing(topk_idx)
    in_maps = []
    for c in range(NCORES):
        sl = src_tok[c * RPC:(c + 1) * RPC]
        idx_arr = np.ascontiguousarray(sl.reshape(NTILES, 128).T)
        in_maps.append({"x": x, "idx": idx_arr,
                        **_common_inputs(x, topk_weights, c)})
    return in_maps, tokens_per_expert


def make_in_maps_dedup(x, topk_idx, topk_weights):
    """Dedup variant inputs. Returns (in_maps, tpe, n_chunks, passes)."""
    x = np.ascontiguousarray(x, dtype=np.float32)
    src_tok, tokens_per_expert = _routing(topk_idx)

    per_core = []
    for c in range(NCORES):
        sl = src_tok[c * RPC:(c + 1) * RPC]
        order = np.argsort(sl, kind="stable")
        u, starts, counts = np.unique(sl[order], return_index=True,
                                      return_counts=True)
        per_core.append((u, order, starts, counts))

    n_chunks = max(int(np.ceil(len(u) / 128)) for u, _, _, _ in per_core)
    passes = []
    for j in range(n_chunks):
        pj = 0
        for u, _, _, counts in per_core:
            cj = counts[j * 128:(j + 1) * 128]
            if len(cj):
                pj = max(pj, int(cj.max()))
        passes.append(pj)
    passes = tuple(passes)
    sp = sum(passes)

    in_maps = []
    for c in range(NCORES):
        u, order, starts, counts = per_core[c]
        n_u = len(u)
        idxg = np.full((128, n_chunks), SENT, dtype=np.int32)
        idxs = np.full((128, sp), SENT, dtype=np.int32)
        col = 0
        for j in range(n_chunks):
            lo = j * 128
            nv = min(128, max(0, n_u - lo))
            if nv > 0:
                idxg[:nv, j] = u[lo:lo + nv]
            for r in range(passes[j]):
                if nv > 0:
                    crange = counts[lo:lo + nv]
                    sel = np.nonzero(crange > r)[0]
                    if len(sel):
                        idxs[sel, col] = order[starts[lo + sel] + r]
                col += 1
        in_maps.append({"x": x, "idxg": np.ascontiguousarray(idxg),
                        "idxs": np.ascontiguousarray(idxs),
                        **_common_inputs(x, topk_weights, c)})
    return in_maps, tokens_per_expert, n_chunks, passes


_module_cache = {}

STRATEGY = "dedup"   # "simple" | "dedup"


def kernel(x, topk_idx, topk_weights):
    if STRATEGY == "dedup":
        in_maps, tpe, n_chunks, passes = make_in_maps_dedup(
            x, topk_idx, topk_weights)
        key = ("dedup", n_chunks, passes)
        if key not in _module_cache:
            _module_cache[key] = build_module_dedup(n_chunks, passes)
    else:
        in_maps, tpe = make_in_maps(x, topk_idx, topk_weights)
        key = ("simple",)
        if key not in _module_cache:
            _module_cache[key] = build_module()
    nc = _module_cache[key]

    res = run_bass_kernel_spmd(nc, in_maps, core_ids=list(range(NCORES)))
    dispatched = np.concatenate([r["disp"] for r in res.results], axis=0)
    combined = np.concatenate([r["comb"] for r in res.results], axis=0)
    return combined, dispatched, tpe


# revision 8
# speedup vs baseline: 1.0640x; 1.0640x over previous
"""MoE all-to-all dispatch + combine (nn_EpAll2AllFusedOp) on 8 trn2 NeuronCores.

Semantics (matching the jax reference):
  flat_expert = topk_idx.reshape(T*K)
  sort_idx    = stable argsort of flat_expert
  dispatched  = x[sort_idx // K]                      # [T*K, H], expert-contiguous
  combined[t] = x[t] * sum_k topk_weights[t, k]      # gather-back of the K copies
  tokens_per_expert = histogram(flat_expert, 64)     # int32

Sharding: the dispatched buffer (T*K = 32768 rows, the 512 MB payload) is
split into 8 equal contiguous row slices - one per core (the expert-parallel
split of the sorted/A2A'd buffer, load-balanced by slot rather than raw
expert id). Each core produces its 4096-row slice of `dispatched` plus its
512-token slice of `combined`. The sort itself is O(T*K) integer metadata
computed on host; all tensor traffic runs on-device.

Two device strategies:
  simple: indirect-gather x[src_token[slot]] per 128-slot tile, direct store.
          ~151 MB/core of DMA traffic.
  dedup:  gather each DISTINCT source row once (~2700 of 4096 slots are
          unique), then indirect-scatter it to all its output slots,
          OOB-masked via bounds_check. ~128 MB/core -> ~15% less traffic.
"""

import numpy as np

import concourse.bass as bass
import concourse.mybir as mybir
import concourse.tile as tile
from concourse import bacc
from concourse.bass_utils import run_bass_kernel_spmd

T = 4096          # tokens
H = 4096          # hidden
K = 8             # topk
E = 64            # experts
NCORES = 8
RPC = T * K // NCORES       # dispatched rows per core = 4096
NTILES = RPC // 128         # gather tiles per core = 32
CTOK = T // NCORES          # combine tokens per core = 512
CTILES = CTOK // 128        # combine tiles per core = 4

FP32 = mybir.dt.float32
I32 = mybir.dt.int32
SENT = T                    # OOB sentinel row index (skipped via bounds_check;
                            # small value so index*row_stride never overflows i32)


def _emit_combine(nc, cpool, meta, w, xc, comb):
    w_sb = meta.tile([128, CTILES * K], FP32, name="w_sb")
    nc.sync.dma_start(out=w_sb[:], in_=w[:])
    ws = [meta.tile([128, 1], FP32, name=f"ws{j}", tag=f"ws{j}")
          for j in range(CTILES)]
    for j in range(CTILES):
        nc.vector.reduce_sum(
            out=ws[j][:], in_=w_sb[:, j * K:(j + 1) * K],
            axis=mybir.AxisListType.X,
        )
    for j in range(CTILES):
        xt = cpool.tile([128, H], FP32, name="xt", tag="c")
        nc.scalar.dma_start(out=xt[:], in_=xc[j * 128:(j + 1) * 128, :])
        nc.vector.tensor_scalar_mul(out=xt[:], in0=xt[:], scalar1=ws[j][:])
        nc.sync.dma_start(out=comb[j * 128:(j + 1) * 128, :], in_=xt[:])


def build_module(gather_bufs: int = 6, comb_bufs: int = 3):
    """Simple variant: one indirect gather + one direct store per 128 slots."""
    nc = bacc.Bacc("TRN2", num_devices=NCORES)
    x = nc.dram_tensor("x", [T, H], FP32, kind="ExternalInput")
    idx = nc.dram_tensor("idx", [128, NTILES], I32, kind="ExternalInput")
    w = nc.dram_tensor("w", [128, CTILES * K], FP32, kind="ExternalInput")
    xc = nc.dram_tensor("xc", [CTOK, H], FP32, kind="ExternalInput")
    disp = nc.dram_tensor("disp", [RPC, H], FP32, kind="ExternalOutput")
    comb = nc.dram_tensor("comb", [CTOK, H], FP32, kind="ExternalOutput")

    with tile.TileContext(nc) as tc:
        with (
            tc.tile_pool(name="meta", bufs=1) as meta,
            tc.tile_pool(name="gpool", bufs=gather_bufs) as gpool,
            tc.tile_pool(name="cpool", bufs=comb_bufs) as cpool,
        ):
            idx_sb = meta.tile([128, NTILES], I32, name="idx_sb")
            nc.sync.dma_start(out=idx_sb[:], in_=idx[:])
            for i in range(NTILES):
                g = gpool.tile([128, H], FP32, name="g", tag="g")
                nc.gpsimd.indirect_dma_start(
                    out=g[:],
                    out_offset=None,
                    in_=x[:],
                    in_offset=bass.IndirectOffsetOnAxis(
                        ap=idx_sb[:, i:i + 1], axis=0),
                )
                nc.sync.dma_start(out=disp[i * 128:(i + 1) * 128, :], in_=g[:])
            _emit_combine(nc, cpool, meta, w, xc, comb)
    nc.compile()
    return nc


def build_module_dedup(n_chunks: int, passes: tuple,
                       gather_bufs: int = 6, comb_bufs: int = 3):
    """Dedup variant: gather distinct rows once, indirect-scatter to all slots.

    n_chunks: number of 128-row unique-token chunks (same on all cores,
    OOB-padded). passes[j]: scatter passes for chunk j (max multiplicity over
    cores; masked slots move no bytes).
    """
    sp = sum(passes)
    nc = bacc.Bacc("TRN2", num_devices=NCORES)
    x = nc.dram_tensor("x", [T, H], FP32, kind="ExternalInput")
    idxg = nc.dram_tensor("idxg", [128, n_chunks], I32, kind="ExternalInput")
    idxs = nc.dram_tensor("idxs", [128, sp], I32, kind="ExternalInput")
    w = nc.dram_tensor("w", [128, CTILES * K], FP32, kind="ExternalInput")
    xc = nc.dram_tensor("xc", [CTOK, H], FP32, kind="ExternalInput")
    disp = nc.dram_tensor("disp", [RPC, H], FP32, kind="ExternalOutput")
    comb = nc.dram_tensor("comb", [CTOK, H], FP32, kind="ExternalOutput")

    with tile.TileContext(nc) as tc:
        with (
            tc.tile_pool(name="meta", bufs=1) as meta,
            tc.tile_pool(name="gpool", bufs=gather_bufs) as gpool,
            tc.tile_pool(name="cpool", bufs=comb_bufs) as cpool,
        ):
            idxg_sb = meta.tile([128, n_chunks], I32, name="idxg_sb")
            nc.sync.dma_start(out=idxg_sb[:], in_=idxg[:])
            idxs_sb = meta.tile([128, sp], I32, name="idxs_sb")
            nc.sync.dma_start(out=idxs_sb[:], in_=idxs[:])
            col = 0
            for j in range(n_chunks):
                g = gpool.tile([128, H], FP32, name="g", tag="g")
                nc.gpsimd.indirect_dma_start(
                    out=g[:],
                    out_offset=None,
                    in_=x[:],
                    in_offset=bass.IndirectOffsetOnAxis(
                        ap=idxg_sb[:, j:j + 1], axis=0),
                    bounds_check=T - 1,
                    oob_is_err=False,
                )
                for _ in range(passes[j]):
                    nc.gpsimd.indirect_dma_start(
                        out=disp[:],
                        out_offset=bass.IndirectOffsetOnAxis(
                            ap=idxs_sb[:, col:col + 1], axis=0),
                        in_=g[:],
                        in_offset=None,
                        bounds_check=RPC - 1,
                        oob_is_err=False,
                    )
                    col += 1
            _emit_combine(nc, cpool, meta, w, xc, comb)
    nc.compile()
    return nc


NCH = 512                   # unique-row chunk size for the gsa strategy
NCOLS = NCH // 16           # int16 index columns per chunk block


def build_module_gsa(n_chunks: int, passes: tuple, comb_bufs: int = 3,
                     scatter_queue: int = 1):
    """Gather/scatter-add variant.

    Each core gathers its distinct source rows once (dma_gather, NCH rows per
    op, ~46 MB instead of 67 MB) and fans each row out to all its output
    slots with dma_scatter_add into the zero-initialized disp buffer
    (CCE-add into zeros == write; every slot is written exactly once).
    The dispatch pipeline lives in a tile_critical section with manual
    semaphores - pure gpsimd, double-buffered, so Tile's conservative
    whole-tensor WAW tracking can't serialize the scatter stream. Per-core
    active-index counts are runtime values loaded into a gpsimd register
    (reg_load) so the SPMD program stays identical across cores.
    """
    n_sc = sum(passes)
    nc = bacc.Bacc("TRN2", num_devices=NCORES,
                   num_swdge_queues=max(2, scatter_queue + 1))
    x = nc.dram_tensor("x", [T, H], FP32, kind="ExternalInput")
    gidx = nc.dram_tensor("gidx", [128, n_chunks * NCOLS], mybir.dt.int16,
                          kind="ExternalInput")
    sidx = nc.dram_tensor("sidx", [128, n_sc * NCOLS], mybir.dt.int16,
                          kind="ExternalInput")
    cnt = nc.dram_tensor("cnt", [1, n_sc], I32, kind="ExternalInput")
    w = nc.dram_tensor("w", [128, CTILES * K], FP32, kind="ExternalInput")
    xc = nc.dram_tensor("xc", [CTOK, H], FP32, kind="ExternalInput")
    disp = nc.dram_tensor("disp", [RPC, H], FP32, kind="ExternalOutput")
    comb = nc.dram_tensor("comb", [CTOK, H], FP32, kind="ExternalOutput")

    gidx_sb = nc.alloc_sbuf_tensor("gidx_sb", [128, n_chunks * NCOLS],
                                   mybir.dt.int16)
    sidx_sb = nc.alloc_sbuf_tensor("sidx_sb", [128, n_sc * NCOLS],
                                   mybir.dt.int16)
    cnt_sb = nc.alloc_sbuf_tensor("cnt_sb", [1, n_sc], I32)
    bufs = [nc.alloc_sbuf_tensor(f"gbuf{b}", [128, NCH // 128, H], FP32)
            for b in range(2)]
    lsem = nc.alloc_semaphore("lsem")
    gsems = [nc.alloc_semaphore("gsem0"), nc.alloc_semaphore("gsem1")]
    ssem = nc.alloc_semaphore("ssem")

    with tile.TileContext(nc) as tc:
        with (
            tc.tile_pool(name="meta", bufs=1) as meta,
            tc.tile_pool(name="cpool", bufs=comb_bufs) as cpool,
        ):
            with tc.tile_critical():
                g = nc.gpsimd
                with g.register("rcnt") as rcnt:
                    g.dma_start(out=gidx_sb[:], in_=gidx[:]).then_inc(lsem, 16)
                    g.dma_start(out=sidx_sb[:], in_=sidx[:]).then_inc(lsem, 16)
                    g.dma_start(out=cnt_sb[:], in_=cnt[:]).then_inc(lsem, 16)
                    g.wait_ge(lsem, 48)

                    def emit_gather(j):
                        g.dma_gather(
                            out_ap=bufs[j % 2][:],
                            in_ap=x[:],
                            idxs_ap=gidx_sb[:, j * NCOLS:(j + 1) * NCOLS],
                            num_idxs=NCH,
                            num_idxs_reg=NCH,
                            elem_size=H,
                            queue_num=0,
                        ).then_inc(gsems[j % 2], 16)

                    for j in range(min(2, n_chunks)):
                        emit_gather(j)
                    col = 0
                    for j in range(n_chunks):
                        g.wait_ge(gsems[j % 2], 16 * (j // 2 + 1))
                        for _ in range(passes[j]):
                            g.reg_load(rcnt, cnt_sb[0:1, col:col + 1])
                            g.dma_scatter_add(
                                out_ap=disp[:],
                                in_ap=bufs[j % 2][:],
                                idxs_ap=sidx_sb[:, col * NCOLS:(col + 1) * NCOLS],
                                num_idxs=NCH,
                                num_idxs_reg=rcnt,
                                elem_size=H,
                                queue_num=scatter_queue,
                            ).then_inc(ssem, 16)
                            col += 1
                        if j + 2 < n_chunks:
                            g.wait_ge(ssem, 16 * col)
                            emit_gather(j + 2)
                    g.wait_ge(ssem, 16 * col)
            _emit_combine(nc, cpool, meta, w, xc, comb)
    nc.compile()
    return nc


def _routing(topk_idx):
    flat = np.ascontiguousarray(topk_idx, dtype=np.int32).reshape(-1)
    sort_idx = np.argsort(flat, kind="stable")
    src_tok = (sort_idx // K).astype(np.int32)
    tokens_per_expert = np.bincount(flat, minlength=E).astype(topk_idx.dtype)
    return src_tok, tokens_per_expert


def _common_inputs(x, topk_weights, c):
    wc = np.ascontiguousarray(topk_weights, dtype=np.float32)[
        c * CTOK:(c + 1) * CTOK]
    w_arr = np.ascontiguousarray(
        wc.reshape(CTILES, 128, K).transpose(1, 0, 2).reshape(128, CTILES * K))
    return {"w": w_arr, "xc": np.ascontiguousarray(x[c * CTOK:(c + 1) * CTOK])}


def make_in_maps(x, topk_idx, topk_weights):
    """Simple variant inputs."""
    x = np.ascontiguousarray(x, dtype=np.float32)
    src_tok, tokens_per_expert = _routing(topk_idx)
    in_maps = []
    for c in range(NCORES):
        sl = src_tok[c * RPC:(c + 1) * RPC]
        idx_arr = np.ascontiguousarray(sl.reshape(NTILES, 128).T)
        in_maps.append({"x": x, "idx": idx_arr,
                        **_common_inputs(x, topk_weights, c)})
    return in_maps, tokens_per_expert


def make_in_maps_dedup(x, topk_idx, topk_weights):
    """Dedup variant inputs. Returns (in_maps, tpe, n_chunks, passes)."""
    x = np.ascontiguousarray(x, dtype=np.float32)
    src_tok, tokens_per_expert = _routing(topk_idx)

    per_core = []
    for c in range(NCORES):
        sl = src_tok[c * RPC:(c + 1) * RPC]
        order = np.argsort(sl, kind="stable")
        u, starts, counts = np.unique(sl[order], return_index=True,
                                      return_counts=True)
        per_core.append((u, order, starts, counts))

    n_chunks = max(int(np.ceil(len(u) / 128)) for u, _, _, _ in per_core)
    passes = []
    for j in range(n_chunks):
        pj = 0
        for u, _, _, counts in per_core:
            cj = counts[j * 128:(j + 1) * 128]
            if len(cj):
                pj = max(pj, int(cj.max()))
        passes.append(pj)
    passes = tuple(passes)
    sp = sum(passes)

    in_maps = []
    for c in range(NCORES):
        u, order, starts, counts = per_core[c]
        n_u = len(u)
        idxg = np.full((128, n_chunks), SENT, dtype=np.int32)
        idxs = np.full((128, sp), SENT, dtype=np.int32)
        col = 0
        for j in range(n_chunks):
            lo = j * 128
            nv = min(128, max(0, n_u - lo))
            if nv > 0:
                idxg[:nv, j] = u[lo:lo + nv]
            for r in range(passes[j]):
                if nv > 0:
                    crange = counts[lo:lo + nv]
                    sel = np.nonzero(crange > r)[0]
                    if len(sel):
                        idxs[sel, col] = order[starts[lo + sel] + r]
                col += 1
        in_maps.append({"x": x, "idxg": np.ascontiguousarray(idxg),
                        "idxs": np.ascontiguousarray(idxs),
                        **_common_inputs(x, topk_weights, c)})
    return in_maps, tokens_per_expert, n_chunks, passes


def make_in_maps_gsa(x, topk_idx, topk_weights):
    """gsa variant inputs. Returns (in_maps, tpe, n_chunks, passes)."""
    x = np.ascontiguousarray(x, dtype=np.float32)
    src_tok, tokens_per_expert = _routing(topk_idx)

    per_core = []
    for c in range(NCORES):
        sl = src_tok[c * RPC:(c + 1) * RPC]
        order = np.argsort(sl, kind="stable")
        u, starts, counts = np.unique(sl[order], return_index=True,
                                      return_counts=True)
        o2 = np.lexsort((u, -counts))   # count desc, token asc
        per_core.append((u[o2], starts[o2], counts[o2], order))

    n_chunks = max(int(np.ceil(len(u) / NCH)) for u, _, _, _ in per_core)
    u_pad = n_chunks * NCH
    passes = []
    for j in range(n_chunks):
        pj = 0
        for _, _, counts, _ in per_core:
            if j * NCH < len(counts):
                pj = max(pj, int(counts[j * NCH]))
        passes.append(pj)
    passes = tuple(passes)
    n_sc = sum(passes)

    in_maps = []
    for c in range(NCORES):
        u2, st2, cn2, order = per_core[c]
        nu = len(u2)
        up = np.full(u_pad, u2[0], dtype=np.int64)
        up[:nu] = u2
        cp = np.zeros(u_pad, dtype=np.int64)
        cp[:nu] = cn2
        sp_ = np.zeros(u_pad, dtype=np.int64)
        sp_[:nu] = st2

        gidx = np.zeros((128, n_chunks * NCOLS), np.int16)
        sidx = np.zeros((128, n_sc * NCOLS), np.int16)
        cnt = np.zeros((1, n_sc), np.int32)

        def put(arr, colblk, vals):
            # idx blocks are wrapped into 16 partitions and replicated for
            # each of the 8 GPSIMD Q7 cores (partition groups of 16)
            blk = vals.astype(np.int16).reshape(NCOLS, 16).T
            for kk in range(8):
                arr[16 * kk:16 * (kk + 1),
                    colblk * NCOLS:(colblk + 1) * NCOLS] = blk

        col = 0
        for j in range(n_chunks):
            put(gidx, j, up[j * NCH:(j + 1) * NCH])
            cj = cp[j * NCH:(j + 1) * NCH]
            stj = sp_[j * NCH:(j + 1) * NCH]
            assert (np.diff(cj) <= 0).all(), "counts must be descending"
            for r in range(passes[j]):
                m = int((cj > r).sum())
                dst = np.full(NCH, -1, np.int64)
                if m:
                    dst[:m] = order[stj[:m] + r]
                put(sidx, col, dst)
                cnt[0, col] = m
                col += 1
        in_maps.append({"x": x, "gidx": gidx, "sidx": sidx, "cnt": cnt,
                        **_common_inputs(x, topk_weights, c)})
    return in_maps, tokens_per_expert, n_chunks, passes


_module_cache = {}

STRATEGY = "gsa"   # "simple" | "dedup" | "gsa"


def kernel(x, topk_idx, topk_weights):
    if STRATEGY == "gsa":
        in_maps, tpe, n_chunks, passes = make_in_maps_gsa(
            x, topk_idx, topk_weights)
        key = ("gsa", n_chunks, passes)
        if key not in _module_cache:
            _module_cache[key] = build_module_gsa(n_chunks, passes)
    elif STRATEGY == "dedup":
        in_maps, tpe, n_chunks, passes = make_in_maps_dedup(
            x, topk_idx, topk_weights)
        key = ("dedup", n_chunks, passes)
        if key not in _module_cache:
            _module_cache[key] = build_module_dedup(n_chunks, passes)
    else:
        in_maps, tpe = make_in_maps(x, topk_idx, topk_weights)
        key = ("simple",)
        if key not in _module_cache:
            _module_cache[key] = build_module()
    nc = _module_cache[key]

    res = run_bass_kernel_spmd(nc, in_maps, core_ids=list(range(NCORES)))
    dispatched = np.concatenate([r["disp"] for r in res.results], axis=0)
    combined = np.concatenate([r["comb"] for r in res.results], axis=0)
    return combined, dispatched, tpe


# revision 10
# speedup vs baseline: 1.5008x; 1.4106x over previous
"""MoE all-to-all dispatch + combine (nn_EpAll2AllFusedOp) on 8 trn2 NeuronCores.

Semantics (matching the jax reference):
  flat_expert = topk_idx.reshape(T*K)
  sort_idx    = stable argsort of flat_expert
  dispatched  = x[sort_idx // K]                      # [T*K, H], expert-contiguous
  combined[t] = x[t] * sum_k topk_weights[t, k]      # gather-back of the K copies
  tokens_per_expert = histogram(flat_expert, 64)     # int32

Sharding: the dispatched buffer (T*K = 32768 rows, the 512 MB payload) is
split into 8 equal contiguous row slices - one per core (the expert-parallel
split of the sorted/A2A'd buffer, load-balanced by slot rather than raw
expert id). Each core produces its 4096-row slice of `dispatched` plus its
512-token slice of `combined`. The sort itself is O(T*K) integer metadata
computed on host; all tensor traffic runs on-device.

Two device strategies:
  simple: indirect-gather x[src_token[slot]] per 128-slot tile, direct store.
          ~151 MB/core of DMA traffic.
  dedup:  gather each DISTINCT source row once (~2700 of 4096 slots are
          unique), then indirect-scatter it to all its output slots,
          OOB-masked via bounds_check. ~128 MB/core -> ~15% less traffic.
"""

import numpy as np

import concourse.bass as bass
import concourse.mybir as mybir
import concourse.tile as tile
from concourse import bacc
from concourse.bass_utils import run_bass_kernel_spmd

T = 4096          # tokens
H = 4096          # hidden
K = 8             # topk
E = 64            # experts
NCORES = 8
RPC = T * K // NCORES       # dispatched rows per core = 4096
NTILES = RPC // 128         # gather tiles per core = 32
CTOK = T // NCORES          # combine tokens per core = 512
CTILES = CTOK // 128        # combine tiles per core = 4

FP32 = mybir.dt.float32
I32 = mybir.dt.int32
SENT = T                    # OOB sentinel row index (skipped via bounds_check;
                            # small value so index*row_stride never overflows i32)


def _emit_combine(nc, cpool, meta, w, xc, comb):
    w_sb = meta.tile([128, CTILES * K], FP32, name="w_sb")
    nc.sync.dma_start(out=w_sb[:], in_=w[:])
    ws = [meta.tile([128, 1], FP32, name=f"ws{j}", tag=f"ws{j}")
          for j in range(CTILES)]
    for j in range(CTILES):
        nc.vector.reduce_sum(
            out=ws[j][:], in_=w_sb[:, j * K:(j + 1) * K],
            axis=mybir.AxisListType.X,
        )
    for j in range(CTILES):
        xt = cpool.tile([128, H], FP32, name="xt", tag="c")
        nc.scalar.dma_start(out=xt[:], in_=xc[j * 128:(j + 1) * 128, :])
        nc.vector.tensor_scalar_mul(out=xt[:], in0=xt[:], scalar1=ws[j][:])
        nc.sync.dma_start(out=comb[j * 128:(j + 1) * 128, :], in_=xt[:])


def build_module(gather_bufs: int = 6, comb_bufs: int = 3):
    """Simple variant: one indirect gather + one direct store per 128 slots."""
    nc = bacc.Bacc("TRN2", num_devices=NCORES)
    x = nc.dram_tensor("x", [T, H], FP32, kind="ExternalInput")
    idx = nc.dram_tensor("idx", [128, NTILES], I32, kind="ExternalInput")
    w = nc.dram_tensor("w", [128, CTILES * K], FP32, kind="ExternalInput")
    xc = nc.dram_tensor("xc", [CTOK, H], FP32, kind="ExternalInput")
    disp = nc.dram_tensor("disp", [RPC, H], FP32, kind="ExternalOutput")
    comb = nc.dram_tensor("comb", [CTOK, H], FP32, kind="ExternalOutput")

    with tile.TileContext(nc) as tc:
        with (
            tc.tile_pool(name="meta", bufs=1) as meta,
            tc.tile_pool(name="gpool", bufs=gather_bufs) as gpool,
            tc.tile_pool(name="cpool", bufs=comb_bufs) as cpool,
        ):
            idx_sb = meta.tile([128, NTILES], I32, name="idx_sb")
            nc.sync.dma_start(out=idx_sb[:], in_=idx[:])
            for i in range(NTILES):
                g = gpool.tile([128, H], FP32, name="g", tag="g")
                nc.gpsimd.indirect_dma_start(
                    out=g[:],
                    out_offset=None,
                    in_=x[:],
                    in_offset=bass.IndirectOffsetOnAxis(
                        ap=idx_sb[:, i:i + 1], axis=0),
                )
                nc.sync.dma_start(out=disp[i * 128:(i + 1) * 128, :], in_=g[:])
            _emit_combine(nc, cpool, meta, w, xc, comb)
    nc.compile()
    return nc


def build_module_dedup(n_chunks: int, passes: tuple,
                       gather_bufs: int = 6, comb_bufs: int = 3):
    """Dedup variant: gather distinct rows once, indirect-scatter to all slots.

    n_chunks: number of 128-row unique-token chunks (same on all cores,
    OOB-padded). passes[j]: scatter passes for chunk j (max multiplicity over
    cores; masked slots move no bytes).
    """
    sp = sum(passes)
    nc = bacc.Bacc("TRN2", num_devices=NCORES)
    x = nc.dram_tensor("x", [T, H], FP32, kind="ExternalInput")
    idxg = nc.dram_tensor("idxg", [128, n_chunks], I32, kind="ExternalInput")
    idxs = nc.dram_tensor("idxs", [128, sp], I32, kind="ExternalInput")
    w = nc.dram_tensor("w", [128, CTILES * K], FP32, kind="ExternalInput")
    xc = nc.dram_tensor("xc", [CTOK, H], FP32, kind="ExternalInput")
    disp = nc.dram_tensor("disp", [RPC, H], FP32, kind="ExternalOutput")
    comb = nc.dram_tensor("comb", [CTOK, H], FP32, kind="ExternalOutput")

    with tile.TileContext(nc) as tc:
        with (
            tc.tile_pool(name="meta", bufs=1) as meta,
            tc.tile_pool(name="gpool", bufs=gather_bufs) as gpool,
            tc.tile_pool(name="cpool", bufs=comb_bufs) as cpool,
        ):
            idxg_sb = meta.tile([128, n_chunks], I32, name="idxg_sb")
            nc.sync.dma_start(out=idxg_sb[:], in_=idxg[:])
            idxs_sb = meta.tile([128, sp], I32, name="idxs_sb")
            nc.sync.dma_start(out=idxs_sb[:], in_=idxs[:])
            col = 0
            for j in range(n_chunks):
                g = gpool.tile([128, H], FP32, name="g", tag="g")
                nc.gpsimd.indirect_dma_start(
                    out=g[:],
                    out_offset=None,
                    in_=x[:],
                    in_offset=bass.IndirectOffsetOnAxis(
                        ap=idxg_sb[:, j:j + 1], axis=0),
                    bounds_check=T - 1,
                    oob_is_err=False,
                )
                for _ in range(passes[j]):
                    nc.gpsimd.indirect_dma_start(
                        out=disp[:],
                        out_offset=bass.IndirectOffsetOnAxis(
                            ap=idxs_sb[:, col:col + 1], axis=0),
                        in_=g[:],
                        in_offset=None,
                        bounds_check=RPC - 1,
                        oob_is_err=False,
                    )
                    col += 1
            _emit_combine(nc, cpool, meta, w, xc, comb)
    nc.compile()
    return nc


NCH = 512                   # unique-row chunk size for the gsa strategy
NCOLS = NCH // 16           # int16 index columns per chunk block


def build_module_gsa(n_chunks: int, passes: tuple, comb_bufs: int = 3,
                     scatter_queue: int = 1):
    """Gather/scatter-add variant.

    Each core gathers its distinct source rows once (dma_gather, NCH rows per
    op, ~46 MB instead of 67 MB) and fans each row out to all its output
    slots with dma_scatter_add into the zero-initialized disp buffer
    (CCE-add into zeros == write; every slot is written exactly once).
    The dispatch pipeline lives in a tile_critical section with manual
    semaphores - pure gpsimd, double-buffered, so Tile's conservative
    whole-tensor WAW tracking can't serialize the scatter stream. Per-core
    active-index counts are runtime values loaded into a gpsimd register
    (reg_load) so the SPMD program stays identical across cores.
    """
    n_sc = sum(passes)
    nc = bacc.Bacc("TRN2", num_devices=NCORES,
                   num_swdge_queues=max(2, scatter_queue + 1))
    x = nc.dram_tensor("x", [T, H], FP32, kind="ExternalInput")
    gidx = nc.dram_tensor("gidx", [128, n_chunks * NCOLS], mybir.dt.int16,
                          kind="ExternalInput")
    sidx = nc.dram_tensor("sidx", [128, n_sc * NCOLS], mybir.dt.int16,
                          kind="ExternalInput")
    cnt = nc.dram_tensor("cnt", [1, n_sc], I32, kind="ExternalInput")
    w = nc.dram_tensor("w", [128, CTILES * K], FP32, kind="ExternalInput")
    xc = nc.dram_tensor("xc", [CTOK, H], FP32, kind="ExternalInput")
    disp = nc.dram_tensor("disp", [RPC, H], FP32, kind="ExternalOutput")
    comb = nc.dram_tensor("comb", [CTOK, H], FP32, kind="ExternalOutput")

    gidx_sb = nc.alloc_sbuf_tensor("gidx_sb", [128, n_chunks * NCOLS],
                                   mybir.dt.int16)
    sidx_sb = nc.alloc_sbuf_tensor("sidx_sb", [128, n_sc * NCOLS],
                                   mybir.dt.int16)
    cnt_sb = nc.alloc_sbuf_tensor("cnt_sb", [1, n_sc], I32)
    bufs = [nc.alloc_sbuf_tensor(f"gbuf{b}", [128, NCH // 128, H], FP32)
            for b in range(2)]
    lsem = nc.alloc_semaphore("lsem")
    gsems = [nc.alloc_semaphore("gsem0"), nc.alloc_semaphore("gsem1")]
    ssem = nc.alloc_semaphore("ssem")

    with tile.TileContext(nc) as tc:
        with (
            tc.tile_pool(name="meta", bufs=1) as meta,
            tc.tile_pool(name="cpool", bufs=comb_bufs) as cpool,
        ):
            with tc.tile_critical():
                g = nc.gpsimd
                with g.register("rcnt") as rcnt:
                    g.dma_start(out=gidx_sb[:], in_=gidx[:]).then_inc(lsem, 16)
                    g.dma_start(out=sidx_sb[:], in_=sidx[:]).then_inc(lsem, 16)
                    g.dma_start(out=cnt_sb[:], in_=cnt[:]).then_inc(lsem, 16)
                    g.wait_ge(lsem, 48)

                    def emit_gather(j):
                        g.dma_gather(
                            out_ap=bufs[j % 2][:],
                            in_ap=x[:],
                            idxs_ap=gidx_sb[:, j * NCOLS:(j + 1) * NCOLS],
                            num_idxs=NCH,
                            num_idxs_reg=NCH,
                            elem_size=H,
                            queue_num=0,
                        ).then_inc(gsems[j % 2], 16)

                    for j in range(min(2, n_chunks)):
                        emit_gather(j)
                    col = 0
                    for j in range(n_chunks):
                        g.wait_ge(gsems[j % 2], 16 * (j // 2 + 1))
                        for _ in range(passes[j]):
                            g.reg_load(rcnt, cnt_sb[0:1, col:col + 1])
                            g.dma_scatter_add(
                                out_ap=disp[:],
                                in_ap=bufs[j % 2][:],
                                idxs_ap=sidx_sb[:, col * NCOLS:(col + 1) * NCOLS],
                                num_idxs=NCH,
                                num_idxs_reg=rcnt,
                                elem_size=H,
                                queue_num=scatter_queue,
                            ).then_inc(ssem, 16)
                            col += 1
                        if j + 2 < n_chunks:
                            g.wait_ge(ssem, 16 * col)
                            emit_gather(j + 2)
                    g.wait_ge(ssem, 16 * col)
            _emit_combine(nc, cpool, meta, w, xc, comb)
    nc.compile()
    return nc


NCH2 = 256                  # unique-row chunk size for dedup2
NCOLS2 = NCH2 // 16
NBUF2 = 3                   # gather buffers (pipeline depth)


def build_module_dedup2(n_chunks: int, passes: tuple, comb_bufs: int = 3):
    """Dedup v2: coalescing dma_gather of unique rows (queue 0) + plain
    indirect-DMA scatter of each row to its output slots (bypass write,
    OOB-masked via bounds_check, queue 1). Manual-semaphore pipeline inside
    a gpsimd-only critical section so Tile's whole-tensor WAW tracking can't
    serialize the scatter stream. passes[j] is a tuple of per-128-column
    scatter pass counts for chunk j (max over cores).
    """
    n_sc = sum(sum(p) for p in passes)
    nc = bacc.Bacc("TRN2", num_devices=NCORES, num_swdge_queues=2)
    x = nc.dram_tensor("x", [T, H], FP32, kind="ExternalInput")
    gidx = nc.dram_tensor("gidx", [128, n_chunks * NCOLS2], mybir.dt.int16,
                          kind="ExternalInput")
    sidx = nc.dram_tensor("sidx", [128, n_sc], I32, kind="ExternalInput")
    w = nc.dram_tensor("w", [128, CTILES * K], FP32, kind="ExternalInput")
    xc = nc.dram_tensor("xc", [CTOK, H], FP32, kind="ExternalInput")
    disp = nc.dram_tensor("disp", [RPC, H], FP32, kind="ExternalOutput")
    comb = nc.dram_tensor("comb", [CTOK, H], FP32, kind="ExternalOutput")

    gidx_sb = nc.alloc_sbuf_tensor("gidx_sb", [128, n_chunks * NCOLS2],
                                   mybir.dt.int16)
    sidx_sb = nc.alloc_sbuf_tensor("sidx_sb", [128, n_sc], I32)
    bufs = [nc.alloc_sbuf_tensor(f"gbuf{b}", [128, NCH2 // 128, H], FP32)
            for b in range(NBUF2)]
    lsem = nc.alloc_semaphore("lsem")
    gsems = [nc.alloc_semaphore(f"gsem{b}") for b in range(NBUF2)]
    ssem = nc.alloc_semaphore("ssem")

    with tile.TileContext(nc) as tc:
        with (
            tc.tile_pool(name="meta", bufs=1) as meta,
            tc.tile_pool(name="cpool", bufs=comb_bufs) as cpool,
        ):
            with tc.tile_critical():
                g = nc.gpsimd
                g.dma_start(out=gidx_sb[:], in_=gidx[:]).then_inc(lsem, 16)
                g.dma_start(out=sidx_sb[:], in_=sidx[:]).then_inc(lsem, 16)
                g.wait_ge(lsem, 32)

                def emit_gather(j):
                    g.dma_gather(
                        out_ap=bufs[j % NBUF2][:],
                        in_ap=x[:],
                        idxs_ap=gidx_sb[:, j * NCOLS2:(j + 1) * NCOLS2],
                        num_idxs=NCH2,
                        num_idxs_reg=NCH2,
                        elem_size=H,
                        queue_num=0,
                    ).then_inc(gsems[j % NBUF2], 16)

                for j in range(min(NBUF2, n_chunks)):
                    emit_gather(j)
                col = 0
                for j in range(n_chunks):
                    g.wait_ge(gsems[j % NBUF2], 16 * (j // NBUF2 + 1))
                    for c in range(NCH2 // 128):
                        for _ in range(passes[j][c]):
                            inst = g.indirect_dma_start(
                                out=disp[:],
                                out_offset=bass.IndirectOffsetOnAxis(
                                    ap=sidx_sb[:, col:col + 1], axis=0),
                                in_=bufs[j % NBUF2][:, c, :],
                                in_offset=None,
                                bounds_check=RPC - 1,
                                oob_is_err=False,
                            )
                            inst.ins.queue = "qPoolDynamic1"
                            inst.then_inc(ssem, 16)
                            col += 1
                    if j + NBUF2 < n_chunks:
                        g.wait_ge(ssem, 16 * col)
                        emit_gather(j + NBUF2)
                g.wait_ge(ssem, 16 * col)
            _emit_combine(nc, cpool, meta, w, xc, comb)
    nc.compile()
    return nc


def make_in_maps_dedup2(x, topk_idx, topk_weights):
    """dedup2 inputs. Returns (in_maps, tpe, n_chunks, passes)."""
    x = np.ascontiguousarray(x, dtype=np.float32)
    src_tok, tokens_per_expert = _routing(topk_idx)

    per_core = []
    for c in range(NCORES):
        sl = src_tok[c * RPC:(c + 1) * RPC]
        order = np.argsort(sl, kind="stable")
        u, starts, counts = np.unique(sl[order], return_index=True,
                                      return_counts=True)
        o2 = np.lexsort((u, -counts))   # count desc, token asc
        per_core.append((u[o2], starts[o2], counts[o2], order))

    n_chunks = max(int(np.ceil(len(u) / NCH2)) for u, _, _, _ in per_core)
    u_pad = n_chunks * NCH2
    passes = []
    for j in range(n_chunks):
        pj = []
        for c128 in range(NCH2 // 128):
            pos = j * NCH2 + c128 * 128
            p = 0
            for _, _, counts, _ in per_core:
                if pos < len(counts):
                    p = max(p, int(counts[pos]))
            pj.append(p)
        passes.append(tuple(pj))
    passes = tuple(passes)
    n_sc = sum(sum(p) for p in passes)

    in_maps = []
    for c in range(NCORES):
        u2, st2, cn2, order = per_core[c]
        nu = len(u2)
        up = np.full(u_pad, u2[0], dtype=np.int64)
        up[:nu] = u2
        cp = np.zeros(u_pad, dtype=np.int64)
        cp[:nu] = cn2
        sp_ = np.zeros(u_pad, dtype=np.int64)
        sp_[:nu] = st2

        gidx = np.zeros((128, n_chunks * NCOLS2), np.int16)
        sidx = np.full((128, n_sc), SENT, np.int32)
        col = 0
        for j in range(n_chunks):
            blk = up[j * NCH2:(j + 1) * NCH2].astype(np.int16)
            wrapped = blk.reshape(NCOLS2, 16).T
            for kk in range(8):
                gidx[16 * kk:16 * (kk + 1),
                     j * NCOLS2:(j + 1) * NCOLS2] = wrapped
            for c128 in range(NCH2 // 128):
                base = j * NCH2 + c128 * 128
                cj = cp[base:base + 128]
                stj = sp_[base:base + 128]
                for r in range(passes[j][c128]):
                    sel = np.nonzero(cj > r)[0]
                    if len(sel):
                        sidx[sel, col] = order[stj[sel] + r]
                    col += 1
        in_maps.append({"x": x, "gidx": gidx, "sidx": sidx,
                        **_common_inputs(x, topk_weights, c)})
    return in_maps, tokens_per_expert, n_chunks, passes


def _routing(topk_idx):
    flat = np.ascontiguousarray(topk_idx, dtype=np.int32).reshape(-1)
    sort_idx = np.argsort(flat, kind="stable")
    src_tok = (sort_idx // K).astype(np.int32)
    tokens_per_expert = np.bincount(flat, minlength=E).astype(topk_idx.dtype)
    return src_tok, tokens_per_expert


def _common_inputs(x, topk_weights, c):
    wc = np.ascontiguousarray(topk_weights, dtype=np.float32)[
        c * CTOK:(c + 1) * CTOK]
    w_arr = np.ascontiguousarray(
        wc.reshape(CTILES, 128, K).transpose(1, 0, 2).reshape(128, CTILES * K))
    return {"w": w_arr, "xc": np.ascontiguousarray(x[c * CTOK:(c + 1) * CTOK])}


def make_in_maps(x, topk_idx, topk_weights):
    """Simple variant inputs."""
    x = np.ascontiguousarray(x, dtype=np.float32)
    src_tok, tokens_per_expert = _routing(topk_idx)
    in_maps = []
    for c in range(NCORES):
        sl = src_tok[c * RPC:(c + 1) * RPC]
        idx_arr = np.ascontiguousarray(sl.reshape(NTILES, 128).T)
        in_maps.append({"x": x, "idx": idx_arr,
                        **_common_inputs(x, topk_weights, c)})
    return in_maps, tokens_per_expert


def make_in_maps_dedup(x, topk_idx, topk_weights):
    """Dedup variant inputs. Returns (in_maps, tpe, n_chunks, passes)."""
    x = np.ascontiguousarray(x, dtype=np.float32)
    src_tok, tokens_per_expert = _routing(topk_idx)

    per_core = []
    for c in range(NCORES):
        sl = src_tok[c * RPC:(c + 1) * RPC]
        order = np.argsort(sl, kind="stable")
        u, starts, counts = np.unique(sl[order], return_index=True,
                                      return_counts=True)
        per_core.append((u, order, starts, counts))

    n_chunks = max(int(np.ceil(len(u) / 128)) for u, _, _, _ in per_core)
    passes = []
    for j in range(n_chunks):
        pj = 0
        for u, _, _, counts in per_core:
            cj = counts[j * 128:(j + 1) * 128]
            if len(cj):
                pj = max(pj, int(cj.max()))
        passes.append(pj)
    passes = tuple(passes)
    sp = sum(passes)

    in_maps = []
    for c in range(NCORES):
        u, order, starts, counts = per_core[c]
        n_u = len(u)
        idxg = np.full((128, n_chunks), SENT, dtype=np.int32)
        idxs = np.full((128, sp), SENT, dtype=np.int32)
        col = 0
        for j in range(n_chunks):
            lo = j * 128
            nv = min(128, max(0, n_u - lo))
            if nv > 0:
                idxg[:nv, j] = u[lo:lo + nv]
            for r in range(passes[j]):
                if nv > 0:
                    crange = counts[lo:lo + nv]
                    sel = np.nonzero(crange > r)[0]
                    if len(sel):
                        idxs[sel, col] = order[starts[lo + sel] + r]
                col += 1
        in_maps.append({"x": x, "idxg": np.ascontiguousarray(idxg),
                        "idxs": np.ascontiguousarray(idxs),
                        **_common_inputs(x, topk_weights, c)})
    return in_maps, tokens_per_expert, n_chunks, passes


def make_in_maps_gsa(x, topk_idx, topk_weights):
    """gsa variant inputs. Returns (in_maps, tpe, n_chunks, passes)."""
    x = np.ascontiguousarray(x, dtype=np.float32)
    src_tok, tokens_per_expert = _routing(topk_idx)

    per_core = []
    for c in range(NCORES):
        sl = src_tok[c * RPC:(c + 1) * RPC]
        order = np.argsort(sl, kind="stable")
        u, starts, counts = np.unique(sl[order], return_index=True,
                                      return_counts=True)
        o2 = np.lexsort((u, -counts))   # count desc, token asc
        per_core.append((u[o2], starts[o2], counts[o2], order))

    n_chunks = max(int(np.ceil(len(u) / NCH)) for u, _, _, _ in per_core)
    u_pad = n_chunks * NCH
    passes = []
    for j in range(n_chunks):
        pj = 0
        for _, _, counts, _ in per_core:
            if j * NCH < len(counts):
                pj = max(pj, int(counts[j * NCH]))
        passes.append(pj)
    passes = tuple(passes)
    n_sc = sum(passes)

    in_maps = []
    for c in range(NCORES):
        u2, st2, cn2, order = per_core[c]
        nu = len(u2)
        up = np.full(u_pad, u2[0], dtype=np.int64)
        up[:nu] = u2
        cp = np.zeros(u_pad, dtype=np.int64)
        cp[:nu] = cn2
        sp_ = np.zeros(u_pad, dtype=np.int64)
        sp_[:nu] = st2

        gidx = np.zeros((128, n_chunks * NCOLS), np.int16)
        sidx = np.zeros((128, n_sc * NCOLS), np.int16)
        cnt = np.zeros((1, n_sc), np.int32)

        def put(arr, colblk, vals):
            # idx blocks are wrapped into 16 partitions and replicated for
            # each of the 8 GPSIMD Q7 cores (partition groups of 16)
            blk = vals.astype(np.int16).reshape(NCOLS, 16).T
            for kk in range(8):
                arr[16 * kk:16 * (kk + 1),
                    colblk * NCOLS:(colblk + 1) * NCOLS] = blk

        col = 0
        for j in range(n_chunks):
            put(gidx, j, up[j * NCH:(j + 1) * NCH])
            cj = cp[j * NCH:(j + 1) * NCH]
            stj = sp_[j * NCH:(j + 1) * NCH]
            assert (np.diff(cj) <= 0).all(), "counts must be descending"
            for r in range(passes[j]):
                m = int((cj > r).sum())
                dst = np.full(NCH, -1, np.int64)
                if m:
                    dst[:m] = order[stj[:m] + r]
                put(sidx, col, dst)
                cnt[0, col] = m
                col += 1
        in_maps.append({"x": x, "gidx": gidx, "sidx": sidx, "cnt": cnt,
                        **_common_inputs(x, topk_weights, c)})
    return in_maps, tokens_per_expert, n_chunks, passes


_module_cache = {}

STRATEGY = "dedup2"   # "simple" | "dedup" | "gsa" | "dedup2"


def kernel(x, topk_idx, topk_weights):
    if STRATEGY == "dedup2":
        in_maps, tpe, n_chunks, passes = make_in_maps_dedup2(
            x, topk_idx, topk_weights)
        key = ("dedup2", n_chunks, passes)
        if key not in _module_cache:
            _module_cache[key] = build_module_dedup2(n_chunks, passes)
    elif STRATEGY == "gsa":
        in_maps, tpe, n_chunks, passes = make_in_maps_gsa(
            x, topk_idx, topk_weights)
        key = ("gsa", n_chunks, passes)
        if key not in _module_cache:
            _module_cache[key] = build_module_gsa(n_chunks, passes)
    elif STRATEGY == "dedup":
        in_maps, tpe, n_chunks, passes = make_in_maps_dedup(
            x, topk_idx, topk_weights)
        key = ("dedup", n_chunks, passes)
        if key not in _module_cache:
            _module_cache[key] = build_module_dedup(n_chunks, passes)
    else:
        in_maps, tpe = make_in_maps(x, topk_idx, topk_weights)
        key = ("simple",)
        if key not in _module_cache:
            _module_cache[key] = build_module()
    nc = _module_cache[key]

    res = run_bass_kernel_spmd(nc, in_maps, core_ids=list(range(NCORES)))
    dispatched = np.concatenate([r["disp"] for r in res.results], axis=0)
    combined = np.concatenate([r["comb"] for r in res.results], axis=0)
    return combined, dispatched, tpe


# revision 11
# speedup vs baseline: 1.5098x; 1.0060x over previous
"""MoE all-to-all dispatch + combine (nn_EpAll2AllFusedOp) on 8 trn2 NeuronCores.

Semantics (matching the jax reference):
  flat_expert = topk_idx.reshape(T*K)
  sort_idx    = stable argsort of flat_expert
  dispatched  = x[sort_idx // K]                      # [T*K, H], expert-contiguous
  combined[t] = x[t] * sum_k topk_weights[t, k]      # gather-back of the K copies
  tokens_per_expert = histogram(flat_expert, 64)     # int32

Sharding: the dispatched buffer (T*K = 32768 rows, the 512 MB payload) is
split into 8 equal contiguous row slices - one per core (the expert-parallel
split of the sorted/A2A'd buffer, load-balanced by slot rather than raw
expert id). Each core produces its 4096-row slice of `dispatched` plus its
512-token slice of `combined`. The sort itself is O(T*K) integer metadata
computed on host; all tensor traffic runs on-device.

Two device strategies:
  simple: indirect-gather x[src_token[slot]] per 128-slot tile, direct store.
          ~151 MB/core of DMA traffic.
  dedup:  gather each DISTINCT source row once (~2700 of 4096 slots are
          unique), then indirect-scatter it to all its output slots,
          OOB-masked via bounds_check. ~128 MB/core -> ~15% less traffic.
"""

import numpy as np

import concourse.bass as bass
import concourse.mybir as mybir
import concourse.tile as tile
from concourse import bacc
from concourse.bass_utils import run_bass_kernel_spmd

T = 4096          # tokens
H = 4096          # hidden
K = 8             # topk
E = 64            # experts
NCORES = 8
RPC = T * K // NCORES       # dispatched rows per core = 4096
NTILES = RPC // 128         # gather tiles per core = 32
CTOK = T // NCORES          # combine tokens per core = 512
CTILES = CTOK // 128        # combine tiles per core = 4

FP32 = mybir.dt.float32
I32 = mybir.dt.int32
SENT = T                    # OOB sentinel row index (skipped via bounds_check;
                            # small value so index*row_stride never overflows i32)


def _emit_combine(nc, cpool, meta, w, xc, comb):
    w_sb = meta.tile([128, CTILES * K], FP32, name="w_sb")
    nc.sync.dma_start(out=w_sb[:], in_=w[:])
    ws = [meta.tile([128, 1], FP32, name=f"ws{j}", tag=f"ws{j}")
          for j in range(CTILES)]
    for j in range(CTILES):
        nc.vector.reduce_sum(
            out=ws[j][:], in_=w_sb[:, j * K:(j + 1) * K],
            axis=mybir.AxisListType.X,
        )
    for j in range(CTILES):
        xt = cpool.tile([128, H], FP32, name="xt", tag="c")
        nc.scalar.dma_start(out=xt[:], in_=xc[j * 128:(j + 1) * 128, :])
        nc.vector.tensor_scalar_mul(out=xt[:], in0=xt[:], scalar1=ws[j][:])
        nc.sync.dma_start(out=comb[j * 128:(j + 1) * 128, :], in_=xt[:])


def build_module(gather_bufs: int = 6, comb_bufs: int = 3):
    """Simple variant: one indirect gather + one direct store per 128 slots."""
    nc = bacc.Bacc("TRN2", num_devices=NCORES)
    x = nc.dram_tensor("x", [T, H], FP32, kind="ExternalInput")
    idx = nc.dram_tensor("idx", [128, NTILES], I32, kind="ExternalInput")
    w = nc.dram_tensor("w", [128, CTILES * K], FP32, kind="ExternalInput")
    xc = nc.dram_tensor("xc", [CTOK, H], FP32, kind="ExternalInput")
    disp = nc.dram_tensor("disp", [RPC, H], FP32, kind="ExternalOutput")
    comb = nc.dram_tensor("comb", [CTOK, H], FP32, kind="ExternalOutput")

    with tile.TileContext(nc) as tc:
        with (
            tc.tile_pool(name="meta", bufs=1) as meta,
            tc.tile_pool(name="gpool", bufs=gather_bufs) as gpool,
            tc.tile_pool(name="cpool", bufs=comb_bufs) as cpool,
        ):
            idx_sb = meta.tile([128, NTILES], I32, name="idx_sb")
            nc.sync.dma_start(out=idx_sb[:], in_=idx[:])
            for i in range(NTILES):
                g = gpool.tile([128, H], FP32, name="g", tag="g")
                nc.gpsimd.indirect_dma_start(
                    out=g[:],
                    out_offset=None,
                    in_=x[:],
                    in_offset=bass.IndirectOffsetOnAxis(
                        ap=idx_sb[:, i:i + 1], axis=0),
                )
                nc.sync.dma_start(out=disp[i * 128:(i + 1) * 128, :], in_=g[:])
            _emit_combine(nc, cpool, meta, w, xc, comb)
    nc.compile()
    return nc


def build_module_dedup(n_chunks: int, passes: tuple,
                       gather_bufs: int = 6, comb_bufs: int = 3):
    """Dedup variant: gather distinct rows once, indirect-scatter to all slots.

    n_chunks: number of 128-row unique-token chunks (same on all cores,
    OOB-padded). passes[j]: scatter passes for chunk j (max multiplicity over
    cores; masked slots move no bytes).
    """
    sp = sum(passes)
    nc = bacc.Bacc("TRN2", num_devices=NCORES)
    x = nc.dram_tensor("x", [T, H], FP32, kind="ExternalInput")
    idxg = nc.dram_tensor("idxg", [128, n_chunks], I32, kind="ExternalInput")
    idxs = nc.dram_tensor("idxs", [128, sp], I32, kind="ExternalInput")
    w = nc.dram_tensor("w", [128, CTILES * K], FP32, kind="ExternalInput")
    xc = nc.dram_tensor("xc", [CTOK, H], FP32, kind="ExternalInput")
    disp = nc.dram_tensor("disp", [RPC, H], FP32, kind="ExternalOutput")
    comb = nc.dram_tensor("comb", [CTOK, H], FP32, kind="ExternalOutput")

    with tile.TileContext(nc) as tc:
        with (
            tc.tile_pool(name="meta", bufs=1) as meta,
            tc.tile_pool(name="gpool", bufs=gather_bufs) as gpool,
            tc.tile_pool(name="cpool", bufs=comb_bufs) as cpool,
        ):
            idxg_sb = meta.tile([128, n_chunks], I32, name="idxg_sb")
            nc.sync.dma_start(out=idxg_sb[:], in_=idxg[:])
            idxs_sb = meta.tile([128, sp], I32, name="idxs_sb")
            nc.sync.dma_start(out=idxs_sb[:], in_=idxs[:])
            col = 0
            for j in range(n_chunks):
                g = gpool.tile([128, H], FP32, name="g", tag="g")
                nc.gpsimd.indirect_dma_start(
                    out=g[:],
                    out_offset=None,
                    in_=x[:],
                    in_offset=bass.IndirectOffsetOnAxis(
                        ap=idxg_sb[:, j:j + 1], axis=0),
                    bounds_check=T - 1,
                    oob_is_err=False,
                )
                for _ in range(passes[j]):
                    nc.gpsimd.indirect_dma_start(
                        out=disp[:],
                        out_offset=bass.IndirectOffsetOnAxis(
                            ap=idxs_sb[:, col:col + 1], axis=0),
                        in_=g[:],
                        in_offset=None,
                        bounds_check=RPC - 1,
                        oob_is_err=False,
                    )
                    col += 1
            _emit_combine(nc, cpool, meta, w, xc, comb)
    nc.compile()
    return nc


NCH = 512                   # unique-row chunk size for the gsa strategy
NCOLS = NCH // 16           # int16 index columns per chunk block


def build_module_gsa(n_chunks: int, passes: tuple, comb_bufs: int = 3,
                     scatter_queue: int = 1):
    """Gather/scatter-add variant.

    Each core gathers its distinct source rows once (dma_gather, NCH rows per
    op, ~46 MB instead of 67 MB) and fans each row out to all its output
    slots with dma_scatter_add into the zero-initialized disp buffer
    (CCE-add into zeros == write; every slot is written exactly once).
    The dispatch pipeline lives in a tile_critical section with manual
    semaphores - pure gpsimd, double-buffered, so Tile's conservative
    whole-tensor WAW tracking can't serialize the scatter stream. Per-core
    active-index counts are runtime values loaded into a gpsimd register
    (reg_load) so the SPMD program stays identical across cores.
    """
    n_sc = sum(passes)
    nc = bacc.Bacc("TRN2", num_devices=NCORES,
                   num_swdge_queues=max(2, scatter_queue + 1))
    x = nc.dram_tensor("x", [T, H], FP32, kind="ExternalInput")
    gidx = nc.dram_tensor("gidx", [128, n_chunks * NCOLS], mybir.dt.int16,
                          kind="ExternalInput")
    sidx = nc.dram_tensor("sidx", [128, n_sc * NCOLS], mybir.dt.int16,
                          kind="ExternalInput")
    cnt = nc.dram_tensor("cnt", [1, n_sc], I32, kind="ExternalInput")
    w = nc.dram_tensor("w", [128, CTILES * K], FP32, kind="ExternalInput")
    xc = nc.dram_tensor("xc", [CTOK, H], FP32, kind="ExternalInput")
    disp = nc.dram_tensor("disp", [RPC, H], FP32, kind="ExternalOutput")
    comb = nc.dram_tensor("comb", [CTOK, H], FP32, kind="ExternalOutput")

    gidx_sb = nc.alloc_sbuf_tensor("gidx_sb", [128, n_chunks * NCOLS],
                                   mybir.dt.int16)
    sidx_sb = nc.alloc_sbuf_tensor("sidx_sb", [128, n_sc * NCOLS],
                                   mybir.dt.int16)
    cnt_sb = nc.alloc_sbuf_tensor("cnt_sb", [1, n_sc], I32)
    bufs = [nc.alloc_sbuf_tensor(f"gbuf{b}", [128, NCH // 128, H], FP32)
            for b in range(2)]
    lsem = nc.alloc_semaphore("lsem")
    gsems = [nc.alloc_semaphore("gsem0"), nc.alloc_semaphore("gsem1")]
    ssem = nc.alloc_semaphore("ssem")

    with tile.TileContext(nc) as tc:
        with (
            tc.tile_pool(name="meta", bufs=1) as meta,
            tc.tile_pool(name="cpool", bufs=comb_bufs) as cpool,
        ):
            with tc.tile_critical():
                g = nc.gpsimd
                with g.register("rcnt") as rcnt:
                    g.dma_start(out=gidx_sb[:], in_=gidx[:]).then_inc(lsem, 16)
                    g.dma_start(out=sidx_sb[:], in_=sidx[:]).then_inc(lsem, 16)
                    g.dma_start(out=cnt_sb[:], in_=cnt[:]).then_inc(lsem, 16)
                    g.wait_ge(lsem, 48)

                    def emit_gather(j):
                        g.dma_gather(
                            out_ap=bufs[j % 2][:],
                            in_ap=x[:],
                            idxs_ap=gidx_sb[:, j * NCOLS:(j + 1) * NCOLS],
                            num_idxs=NCH,
                            num_idxs_reg=NCH,
                            elem_size=H,
                            queue_num=0,
                        ).then_inc(gsems[j % 2], 16)

                    for j in range(min(2, n_chunks)):
                        emit_gather(j)
                    col = 0
                    for j in range(n_chunks):
                        g.wait_ge(gsems[j % 2], 16 * (j // 2 + 1))
                        for _ in range(passes[j]):
                            g.reg_load(rcnt, cnt_sb[0:1, col:col + 1])
                            g.dma_scatter_add(
                                out_ap=disp[:],
                                in_ap=bufs[j % 2][:],
                                idxs_ap=sidx_sb[:, col * NCOLS:(col + 1) * NCOLS],
                                num_idxs=NCH,
                                num_idxs_reg=rcnt,
                                elem_size=H,
                                queue_num=scatter_queue,
                            ).then_inc(ssem, 16)
                            col += 1
                        if j + 2 < n_chunks:
                            g.wait_ge(ssem, 16 * col)
                            emit_gather(j + 2)
                    g.wait_ge(ssem, 16 * col)
            _emit_combine(nc, cpool, meta, w, xc, comb)
    nc.compile()
    return nc


NCH2 = 256                  # unique-row chunk size for dedup2
NCOLS2 = NCH2 // 16
NBUF2 = 4                   # gather buffers (pipeline depth)


def build_module_dedup2(n_chunks: int, passes: tuple, comb_bufs: int = 3):
    """Dedup v2: coalescing dma_gather of unique rows (queue 0) + plain
    indirect-DMA scatter of each row to its output slots (bypass write,
    OOB-masked via bounds_check, queue 1). Manual-semaphore pipeline inside
    a gpsimd-only critical section so Tile's whole-tensor WAW tracking can't
    serialize the scatter stream. passes[j] is a tuple of per-128-column
    scatter pass counts for chunk j (max over cores).
    """
    n_sc = sum(sum(p) for p in passes)
    nc = bacc.Bacc("TRN2", num_devices=NCORES, num_swdge_queues=2)
    x = nc.dram_tensor("x", [T, H], FP32, kind="ExternalInput")
    gidx = nc.dram_tensor("gidx", [128, n_chunks * NCOLS2], mybir.dt.int16,
                          kind="ExternalInput")
    sidx = nc.dram_tensor("sidx", [128, n_sc], I32, kind="ExternalInput")
    w = nc.dram_tensor("w", [128, CTILES * K], FP32, kind="ExternalInput")
    xc = nc.dram_tensor("xc", [CTOK, H], FP32, kind="ExternalInput")
    disp = nc.dram_tensor("disp", [RPC, H], FP32, kind="ExternalOutput")
    comb = nc.dram_tensor("comb", [CTOK, H], FP32, kind="ExternalOutput")

    gidx_sb = nc.alloc_sbuf_tensor("gidx_sb", [128, n_chunks * NCOLS2],
                                   mybir.dt.int16)
    sidx_sb = nc.alloc_sbuf_tensor("sidx_sb", [128, n_sc], I32)
    bufs = [nc.alloc_sbuf_tensor(f"gbuf{b}", [128, NCH2 // 128, H], FP32)
            for b in range(NBUF2)]
    lsem = nc.alloc_semaphore("lsem")
    gsems = [nc.alloc_semaphore(f"gsem{b}") for b in range(NBUF2)]
    ssem = nc.alloc_semaphore("ssem")

    with tile.TileContext(nc) as tc:
        with (
            tc.tile_pool(name="meta", bufs=1) as meta,
            tc.tile_pool(name="cpool", bufs=comb_bufs) as cpool,
        ):
            with tc.tile_critical():
                g = nc.gpsimd
                g.dma_start(out=gidx_sb[:], in_=gidx[:]).then_inc(lsem, 16)
                g.dma_start(out=sidx_sb[:], in_=sidx[:]).then_inc(lsem, 16)
                g.wait_ge(lsem, 32)

                def emit_gather(j):
                    g.dma_gather(
                        out_ap=bufs[j % NBUF2][:],
                        in_ap=x[:],
                        idxs_ap=gidx_sb[:, j * NCOLS2:(j + 1) * NCOLS2],
                        num_idxs=NCH2,
                        num_idxs_reg=NCH2,
                        elem_size=H,
                        queue_num=1,
                    ).then_inc(gsems[j % NBUF2], 16)

                for j in range(min(NBUF2, n_chunks)):
                    emit_gather(j)
                col = 0
                for j in range(n_chunks):
                    g.wait_ge(gsems[j % NBUF2], 16 * (j // NBUF2 + 1))
                    for c in range(NCH2 // 128):
                        for _ in range(passes[j][c]):
                            inst = g.indirect_dma_start(
                                out=disp[:],
                                out_offset=bass.IndirectOffsetOnAxis(
                                    ap=sidx_sb[:, col:col + 1], axis=0),
                                in_=bufs[j % NBUF2][:, c, :],
                                in_offset=None,
                                bounds_check=RPC - 1,
                                oob_is_err=False,
                            )
                            inst.then_inc(ssem, 16)
                            col += 1
                    if j + NBUF2 < n_chunks:
                        g.wait_ge(ssem, 16 * col)
                        emit_gather(j + NBUF2)
                g.wait_ge(ssem, 16 * col)
            _emit_combine(nc, cpool, meta, w, xc, comb)
    nc.compile()
    return nc


def make_in_maps_dedup2(x, topk_idx, topk_weights):
    """dedup2 inputs. Returns (in_maps, tpe, n_chunks, passes)."""
    x = np.ascontiguousarray(x, dtype=np.float32)
    src_tok, tokens_per_expert = _routing(topk_idx)

    per_core = []
    for c in range(NCORES):
        sl = src_tok[c * RPC:(c + 1) * RPC]
        order = np.argsort(sl, kind="stable")
        u, starts, counts = np.unique(sl[order], return_index=True,
                                      return_counts=True)
        o2 = np.lexsort((u, -counts))   # count desc, token asc
        per_core.append((u[o2], starts[o2], counts[o2], order))

    n_chunks = max(int(np.ceil(len(u) / NCH2)) for u, _, _, _ in per_core)
    u_pad = n_chunks * NCH2
    passes = []
    for j in range(n_chunks):
        pj = []
        for c128 in range(NCH2 // 128):
            pos = j * NCH2 + c128 * 128
            p = 0
            for _, _, counts, _ in per_core:
                if pos < len(counts):
                    p = max(p, int(counts[pos]))
            pj.append(p)
        passes.append(tuple(pj))
    passes = tuple(passes)
    n_sc = sum(sum(p) for p in passes)

    in_maps = []
    for c in range(NCORES):
        u2, st2, cn2, order = per_core[c]
        nu = len(u2)
        up = np.full(u_pad, u2[0], dtype=np.int64)
        up[:nu] = u2
        cp = np.zeros(u_pad, dtype=np.int64)
        cp[:nu] = cn2
        sp_ = np.zeros(u_pad, dtype=np.int64)
        sp_[:nu] = st2

        gidx = np.zeros((128, n_chunks * NCOLS2), np.int16)
        sidx = np.full((128, n_sc), SENT, np.int32)
        col = 0
        for j in range(n_chunks):
            blk = up[j * NCH2:(j + 1) * NCH2].astype(np.int16)
            wrapped = blk.reshape(NCOLS2, 16).T
            for kk in range(8):
                gidx[16 * kk:16 * (kk + 1),
                     j * NCOLS2:(j + 1) * NCOLS2] = wrapped
            for c128 in range(NCH2 // 128):
                base = j * NCH2 + c128 * 128
                cj = cp[base:base + 128]
                stj = sp_[base:base + 128]
                for r in range(passes[j][c128]):
                    sel = np.nonzero(cj > r)[0]
                    if len(sel):
                        sidx[sel, col] = order[stj[sel] + r]
                    col += 1
        in_maps.append({"x": x, "gidx": gidx, "sidx": sidx,
                        **_common_inputs(x, topk_weights, c)})
    return in_maps, tokens_per_expert, n_chunks, passes


def _routing(topk_idx):
    flat = np.ascontiguousarray(topk_idx, dtype=np.int32).reshape(-1)
    sort_idx = np.argsort(flat, kind="stable")
    src_tok = (sort_idx // K).astype(np.int32)
    tokens_per_expert = np.bincount(flat, minlength=E).astype(topk_idx.dtype)
    return src_tok, tokens_per_expert


def _common_inputs(x, topk_weights, c):
    wc = np.ascontiguousarray(topk_weights, dtype=np.float32)[
        c * CTOK:(c + 1) * CTOK]
    w_arr = np.ascontiguousarray(
        wc.reshape(CTILES, 128, K).transpose(1, 0, 2).reshape(128, CTILES * K))
    return {"w": w_arr, "xc": np.ascontiguousarray(x[c * CTOK:(c + 1) * CTOK])}


def make_in_maps(x, topk_idx, topk_weights):
    """Simple variant inputs."""
    x = np.ascontiguousarray(x, dtype=np.float32)
    src_tok, tokens_per_expert = _routing(topk_idx)
    in_maps = []
    for c in range(NCORES):
        sl = src_tok[c * RPC:(c + 1) * RPC]
        idx_arr = np.ascontiguousarray(sl.reshape(NTILES, 128).T)
        in_maps.append({"x": x, "idx": idx_arr,
                        **_common_inputs(x, topk_weights, c)})
    return in_maps, tokens_per_expert


def make_in_maps_dedup(x, topk_idx, topk_weights):
    """Dedup variant inputs. Returns (in_maps, tpe, n_chunks, passes)."""
    x = np.ascontiguousarray(x, dtype=np.float32)
    src_tok, tokens_per_expert = _routing(topk_idx)

    per_core = []
    for c in range(NCORES):
        sl = src_tok[c * RPC:(c + 1) * RPC]
        order = np.argsort(sl, kind="stable")
        u, starts, counts = np.unique(sl[order], return_index=True,
                                      return_counts=True)
        per_core.append((u, order, starts, counts))

    n_chunks = max(int(np.ceil(len(u) / 128)) for u, _, _, _ in per_core)
    passes = []
    for j in range(n_chunks):
        pj = 0
        for u, _, _, counts in per_core:
            cj = counts[j * 128:(j + 1) * 128]
            if len(cj):
                pj = max(pj, int(cj.max()))
        passes.append(pj)
    passes = tuple(passes)
    sp = sum(passes)

    in_maps = []
    for c in range(NCORES):
        u, order, starts, counts = per_core[c]
        n_u = len(u)
        idxg = np.full((128, n_chunks), SENT, dtype=np.int32)
        idxs = np.full((128, sp), SENT, dtype=np.int32)
        col = 0
        for j in range(n_chunks):
            lo = j * 128
            nv = min(128, max(0, n_u - lo))
            if nv > 0:
                idxg[:nv, j] = u[lo:lo + nv]
            for r in range(passes[j]):
                if nv > 0:
                    crange = counts[lo:lo + nv]
                    sel = np.nonzero(crange > r)[0]
                    if len(sel):
                        idxs[sel, col] = order[starts[lo + sel] + r]
                col += 1
        in_maps.append({"x": x, "idxg": np.ascontiguousarray(idxg),
                        "idxs": np.ascontiguousarray(idxs),
                        **_common_inputs(x, topk_weights, c)})
    return in_maps, tokens_per_expert, n_chunks, passes


def make_in_maps_gsa(x, topk_idx, topk_weights):
    """gsa variant inputs. Returns (in_maps, tpe, n_chunks, passes)."""
    x = np.ascontiguousarray(x, dtype=np.float32)
    src_tok, tokens_per_expert = _routing(topk_idx)

    per_core = []
    for c in range(NCORES):
        sl = src_tok[c * RPC:(c + 1) * RPC]
        order = np.argsort(sl, kind="stable")
        u, starts, counts = np.unique(sl[order], return_index=True,
                                      return_counts=True)
        o2 = np.lexsort((u, -counts))   # count desc, token asc
        per_core.append((u[o2], starts[o2], counts[o2], order))

    n_chunks = max(int(np.ceil(len(u) / NCH)) for u, _, _, _ in per_core)
    u_pad = n_chunks * NCH
    passes = []
    for j in range(n_chunks):
        pj = 0
        for _, _, counts, _ in per_core:
            if j * NCH < len(counts):
                pj = max(pj, int(counts[j * NCH]))
        passes.append(pj)
    passes = tuple(passes)
    n_sc = sum(passes)

    in_maps = []
    for c in range(NCORES):
        u2, st2, cn2, order = per_core[c]
        nu = len(u2)
        up = np.full(u_pad, u2[0], dtype=np.int64)
        up[:nu] = u2
        cp = np.zeros(u_pad, dtype=np.int64)
        cp[:nu] = cn2
        sp_ = np.zeros(u_pad, dtype=np.int64)
        sp_[:nu] = st2

        gidx = np.zeros((128, n_chunks * NCOLS), np.int16)
        sidx = np.zeros((128, n_sc * NCOLS), np.int16)
        cnt = np.zeros((1, n_sc), np.int32)

        def put(arr, colblk, vals):
            # idx blocks are wrapped into 16 partitions and replicated for
            # each of the 8 GPSIMD Q7 cores (partition groups of 16)
            blk = vals.astype(np.int16).reshape(NCOLS, 16).T
            for kk in range(8):
                arr[16 * kk:16 * (kk + 1),
                    colblk * NCOLS:(colblk + 1) * NCOLS] = blk

        col = 0
        for j in range(n_chunks):
            put(gidx, j, up[j * NCH:(j + 1) * NCH])
            cj = cp[j * NCH:(j + 1) * NCH]
            stj = sp_[j * NCH:(j + 1) * NCH]
            assert (np.diff(cj) <= 0).all(), "counts must be descending"
            for r in range(passes[j]):
                m = int((cj > r).sum())
                dst = np.full(NCH, -1, np.int64)
                if m:
                    dst[:m] = order[stj[:m] + r]
                put(sidx, col, dst)
                cnt[0, col] = m
                col += 1
        in_maps.append({"x": x, "gidx": gidx, "sidx": sidx, "cnt": cnt,
                        **_common_inputs(x, topk_weights, c)})
    return in_maps, tokens_per_expert, n_chunks, passes


_module_cache = {}

STRATEGY = "dedup2"   # "simple" | "dedup" | "gsa" | "dedup2"


def kernel(x, topk_idx, topk_weights):
    if STRATEGY == "dedup2":
        in_maps, tpe, n_chunks, passes = make_in_maps_dedup2(
            x, topk_idx, topk_weights)
        key = ("dedup2", n_chunks, passes)
        if key not in _module_cache:
            _module_cache[key] = build_module_dedup2(n_chunks, passes)
    elif STRATEGY == "gsa":
        in_maps, tpe, n_chunks, passes = make_in_maps_gsa(
            x, topk_idx, topk_weights)
        key = ("gsa", n_chunks, passes)
        if key not in _module_cache:
            _module_cache[key] = build_module_gsa(n_chunks, passes)
    elif STRATEGY == "dedup":
        in_maps, tpe, n_chunks, passes = make_in_maps_dedup(
            x, topk_idx, topk_weights)
        key = ("dedup", n_chunks, passes)
        if key not in _module_cache:
            _module_cache[key] = build_module_dedup(n_chunks, passes)
    else:
        in_maps, tpe = make_in_maps(x, topk_idx, topk_weights)
        key = ("simple",)
        if key not in _module_cache:
            _module_cache[key] = build_module()
    nc = _module_cache[key]

    res = run_bass_kernel_spmd(nc, in_maps, core_ids=list(range(NCORES)))
    dispatched = np.concatenate([r["disp"] for r in res.results], axis=0)
    combined = np.concatenate([r["comb"] for r in res.results], axis=0)
    return combined, dispatched, tpe


# revision 13
# speedup vs baseline: 1.8397x; 1.2185x over previous
"""MoE all-to-all dispatch + combine (nn_EpAll2AllFusedOp) on 8 trn2 NeuronCores.

Semantics (matching the jax reference):
  flat_expert = topk_idx.reshape(T*K)
  sort_idx    = stable argsort of flat_expert
  dispatched  = x[sort_idx // K]                      # [T*K, H], expert-contiguous
  combined[t] = x[t] * sum_k topk_weights[t, k]      # gather-back of the K copies
  tokens_per_expert = histogram(flat_expert, 64)     # int32

Sharding: the dispatched buffer (T*K = 32768 rows, the 512 MB payload) is
split into 8 equal contiguous row slices - one per core (the expert-parallel
split of the sorted/A2A'd buffer, load-balanced by slot rather than raw
expert id). Each core produces its 4096-row slice of `dispatched` plus its
512-token slice of `combined`. The sort itself is O(T*K) integer metadata
computed on host; all tensor traffic runs on-device.

Two device strategies:
  simple: indirect-gather x[src_token[slot]] per 128-slot tile, direct store.
          ~151 MB/core of DMA traffic.
  dedup:  gather each DISTINCT source row once (~2700 of 4096 slots are
          unique), then indirect-scatter it to all its output slots,
          OOB-masked via bounds_check. ~128 MB/core -> ~15% less traffic.
"""

import numpy as np

import concourse.bass as bass
import concourse.mybir as mybir
import concourse.tile as tile
from concourse import bacc
from concourse.bass_utils import run_bass_kernel_spmd

T = 4096          # tokens
H = 4096          # hidden
K = 8             # topk
E = 64            # experts
NCORES = 8
RPC = T * K // NCORES       # dispatched rows per core = 4096
NTILES = RPC // 128         # gather tiles per core = 32
CTOK = T // NCORES          # combine tokens per core = 512
CTILES = CTOK // 128        # combine tiles per core = 4

FP32 = mybir.dt.float32
I32 = mybir.dt.int32
SENT = T                    # OOB sentinel row index (skipped via bounds_check;
                            # small value so index*row_stride never overflows i32)


def _emit_combine(nc, cpool, meta, w, xc, comb):
    w_sb = meta.tile([128, CTILES * K], FP32, name="w_sb")
    nc.sync.dma_start(out=w_sb[:], in_=w[:])
    ws = [meta.tile([128, 1], FP32, name=f"ws{j}", tag=f"ws{j}")
          for j in range(CTILES)]
    for j in range(CTILES):
        nc.vector.reduce_sum(
            out=ws[j][:], in_=w_sb[:, j * K:(j + 1) * K],
            axis=mybir.AxisListType.X,
        )
    for j in range(CTILES):
        xt = cpool.tile([128, H], FP32, name="xt", tag="c")
        nc.scalar.dma_start(out=xt[:], in_=xc[j * 128:(j + 1) * 128, :])
        nc.vector.tensor_scalar_mul(out=xt[:], in0=xt[:], scalar1=ws[j][:])
        nc.sync.dma_start(out=comb[j * 128:(j + 1) * 128, :], in_=xt[:])


def build_module(gather_bufs: int = 6, comb_bufs: int = 3):
    """Simple variant: one indirect gather + one direct store per 128 slots."""
    nc = bacc.Bacc("TRN2", num_devices=NCORES)
    x = nc.dram_tensor("x", [T, H], FP32, kind="ExternalInput")
    idx = nc.dram_tensor("idx", [128, NTILES], I32, kind="ExternalInput")
    w = nc.dram_tensor("w", [128, CTILES * K], FP32, kind="ExternalInput")
    xc = nc.dram_tensor("xc", [CTOK, H], FP32, kind="ExternalInput")
    disp = nc.dram_tensor("disp", [RPC, H], FP32, kind="ExternalOutput")
    comb = nc.dram_tensor("comb", [CTOK, H], FP32, kind="ExternalOutput")

    with tile.TileContext(nc) as tc:
        with (
            tc.tile_pool(name="meta", bufs=1) as meta,
            tc.tile_pool(name="gpool", bufs=gather_bufs) as gpool,
            tc.tile_pool(name="cpool", bufs=comb_bufs) as cpool,
        ):
            idx_sb = meta.tile([128, NTILES], I32, name="idx_sb")
            nc.sync.dma_start(out=idx_sb[:], in_=idx[:])
            for i in range(NTILES):
                g = gpool.tile([128, H], FP32, name="g", tag="g")
                nc.gpsimd.indirect_dma_start(
                    out=g[:],
                    out_offset=None,
                    in_=x[:],
                    in_offset=bass.IndirectOffsetOnAxis(
                        ap=idx_sb[:, i:i + 1], axis=0),
                )
                nc.sync.dma_start(out=disp[i * 128:(i + 1) * 128, :], in_=g[:])
            _emit_combine(nc, cpool, meta, w, xc, comb)
    nc.compile()
    return nc


def build_module_dedup(n_chunks: int, passes: tuple,
                       gather_bufs: int = 6, comb_bufs: int = 3):
    """Dedup variant: gather distinct rows once, indirect-scatter to all slots.

    n_chunks: number of 128-row unique-token chunks (same on all cores,
    OOB-padded). passes[j]: scatter passes for chunk j (max multiplicity over
    cores; masked slots move no bytes).
    """
    sp = sum(passes)
    nc = bacc.Bacc("TRN2", num_devices=NCORES)
    x = nc.dram_tensor("x", [T, H], FP32, kind="ExternalInput")
    idxg = nc.dram_tensor("idxg", [128, n_chunks], I32, kind="ExternalInput")
    idxs = nc.dram_tensor("idxs", [128, sp], I32, kind="ExternalInput")
    w = nc.dram_tensor("w", [128, CTILES * K], FP32, kind="ExternalInput")
    xc = nc.dram_tensor("xc", [CTOK, H], FP32, kind="ExternalInput")
    disp = nc.dram_tensor("disp", [RPC, H], FP32, kind="ExternalOutput")
    comb = nc.dram_tensor("comb", [CTOK, H], FP32, kind="ExternalOutput")

    with tile.TileContext(nc) as tc:
        with (
            tc.tile_pool(name="meta", bufs=1) as meta,
            tc.tile_pool(name="gpool", bufs=gather_bufs) as gpool,
            tc.tile_pool(name="cpool", bufs=comb_bufs) as cpool,
        ):
            idxg_sb = meta.tile([128, n_chunks], I32, name="idxg_sb")
            nc.sync.dma_start(out=idxg_sb[:], in_=idxg[:])
            idxs_sb = meta.tile([128, sp], I32, name="idxs_sb")
            nc.sync.dma_start(out=idxs_sb[:], in_=idxs[:])
            col = 0
            for j in range(n_chunks):
                g = gpool.tile([128, H], FP32, name="g", tag="g")
                nc.gpsimd.indirect_dma_start(
                    out=g[:],
                    out_offset=None,
                    in_=x[:],
                    in_offset=bass.IndirectOffsetOnAxis(
                        ap=idxg_sb[:, j:j + 1], axis=0),
                    bounds_check=T - 1,
                    oob_is_err=False,
                )
                for _ in range(passes[j]):
                    nc.gpsimd.indirect_dma_start(
                        out=disp[:],
                        out_offset=bass.IndirectOffsetOnAxis(
                            ap=idxs_sb[:, col:col + 1], axis=0),
                        in_=g[:],
                        in_offset=None,
                        bounds_check=RPC - 1,
                        oob_is_err=False,
                    )
                    col += 1
            _emit_combine(nc, cpool, meta, w, xc, comb)
    nc.compile()
    return nc


NCH = 512                   # unique-row chunk size for the gsa strategy
NCOLS = NCH // 16           # int16 index columns per chunk block


def build_module_gsa(n_chunks: int, passes: tuple, comb_bufs: int = 3,
                     scatter_queue: int = 1):
    """Gather/scatter-add variant.

    Each core gathers its distinct source rows once (dma_gather, NCH rows per
    op, ~46 MB instead of 67 MB) and fans each row out to all its output
    slots with dma_scatter_add into the zero-initialized disp buffer
    (CCE-add into zeros == write; every slot is written exactly once).
    The dispatch pipeline lives in a tile_critical section with manual
    semaphores - pure gpsimd, double-buffered, so Tile's conservative
    whole-tensor WAW tracking can't serialize the scatter stream. Per-core
    active-index counts are runtime values loaded into a gpsimd register
    (reg_load) so the SPMD program stays identical across cores.
    """
    n_sc = sum(passes)
    nc = bacc.Bacc("TRN2", num_devices=NCORES,
                   num_swdge_queues=max(2, scatter_queue + 1))
    x = nc.dram_tensor("x", [T, H], FP32, kind="ExternalInput")
    gidx = nc.dram_tensor("gidx", [128, n_chunks * NCOLS], mybir.dt.int16,
                          kind="ExternalInput")
    sidx = nc.dram_tensor("sidx", [128, n_sc * NCOLS], mybir.dt.int16,
                          kind="ExternalInput")
    cnt = nc.dram_tensor("cnt", [1, n_sc], I32, kind="ExternalInput")
    w = nc.dram_tensor("w", [128, CTILES * K], FP32, kind="ExternalInput")
    xc = nc.dram_tensor("xc", [CTOK, H], FP32, kind="ExternalInput")
    disp = nc.dram_tensor("disp", [RPC, H], FP32, kind="ExternalOutput")
    comb = nc.dram_tensor("comb", [CTOK, H], FP32, kind="ExternalOutput")

    gidx_sb = nc.alloc_sbuf_tensor("gidx_sb", [128, n_chunks * NCOLS],
                                   mybir.dt.int16)
    sidx_sb = nc.alloc_sbuf_tensor("sidx_sb", [128, n_sc * NCOLS],
                                   mybir.dt.int16)
    cnt_sb = nc.alloc_sbuf_tensor("cnt_sb", [1, n_sc], I32)
    bufs = [nc.alloc_sbuf_tensor(f"gbuf{b}", [128, NCH // 128, H], FP32)
            for b in range(2)]
    lsem = nc.alloc_semaphore("lsem")
    gsems = [nc.alloc_semaphore("gsem0"), nc.alloc_semaphore("gsem1")]
    ssem = nc.alloc_semaphore("ssem")

    with tile.TileContext(nc) as tc:
        with (
            tc.tile_pool(name="meta", bufs=1) as meta,
            tc.tile_pool(name="cpool", bufs=comb_bufs) as cpool,
        ):
            with tc.tile_critical():
                g = nc.gpsimd
                with g.register("rcnt") as rcnt:
                    g.dma_start(out=gidx_sb[:], in_=gidx[:]).then_inc(lsem, 16)
                    g.dma_start(out=sidx_sb[:], in_=sidx[:]).then_inc(lsem, 16)
                    g.dma_start(out=cnt_sb[:], in_=cnt[:]).then_inc(lsem, 16)
                    g.wait_ge(lsem, 48)

                    def emit_gather(j):
                        g.dma_gather(
                            out_ap=bufs[j % 2][:],
                            in_ap=x[:],
                            idxs_ap=gidx_sb[:, j * NCOLS:(j + 1) * NCOLS],
                            num_idxs=NCH,
                            num_idxs_reg=NCH,
                            elem_size=H,
                            queue_num=0,
                        ).then_inc(gsems[j % 2], 16)

                    for j in range(min(2, n_chunks)):
                        emit_gather(j)
                    col = 0
                    for j in range(n_chunks):
                        g.wait_ge(gsems[j % 2], 16 * (j // 2 + 1))
                        for _ in range(passes[j]):
                            g.reg_load(rcnt, cnt_sb[0:1, col:col + 1])
                            g.dma_scatter_add(
                                out_ap=disp[:],
                                in_ap=bufs[j % 2][:],
                                idxs_ap=sidx_sb[:, col * NCOLS:(col + 1) * NCOLS],
                                num_idxs=NCH,
                                num_idxs_reg=rcnt,
                                elem_size=H,
                                queue_num=scatter_queue,
                            ).then_inc(ssem, 16)
                            col += 1
                        if j + 2 < n_chunks:
                            g.wait_ge(ssem, 16 * col)
                            emit_gather(j + 2)
                    g.wait_ge(ssem, 16 * col)
            _emit_combine(nc, cpool, meta, w, xc, comb)
    nc.compile()
    return nc


NCH2 = 256                  # unique-row chunk size for dedup2
NCOLS2 = NCH2 // 16
NBUF2 = 4                   # gather buffers (pipeline depth)


def build_module_dedup2(n_chunks: int, passes: tuple, comb_bufs: int = 3):
    """Dedup v2: coalescing dma_gather of unique rows (queue 0) + plain
    indirect-DMA scatter of each row to its output slots (bypass write,
    OOB-masked via bounds_check, queue 1). Manual-semaphore pipeline inside
    a gpsimd-only critical section so Tile's whole-tensor WAW tracking can't
    serialize the scatter stream. passes[j] is a tuple of per-128-column
    scatter pass counts for chunk j (max over cores).
    """
    n_sc = sum(sum(p) for p in passes)
    nc = bacc.Bacc("TRN2", num_devices=NCORES, num_swdge_queues=2)
    x = nc.dram_tensor("x", [T, H], FP32, kind="ExternalInput")
    gidx = nc.dram_tensor("gidx", [128, n_chunks * NCOLS2], mybir.dt.int16,
                          kind="ExternalInput")
    sidx = nc.dram_tensor("sidx", [128, n_sc], I32, kind="ExternalInput")
    w = nc.dram_tensor("w", [128, CTILES * K], FP32, kind="ExternalInput")
    xc = nc.dram_tensor("xc", [CTOK, H], FP32, kind="ExternalInput")
    disp = nc.dram_tensor("disp", [RPC, H], FP32, kind="ExternalOutput")
    comb = nc.dram_tensor("comb", [CTOK, H], FP32, kind="ExternalOutput")

    gidx_sb = nc.alloc_sbuf_tensor("gidx_sb", [128, n_chunks * NCOLS2],
                                   mybir.dt.int16)
    sidx_sb = nc.alloc_sbuf_tensor("sidx_sb", [128, n_sc], I32)
    bufs = [nc.alloc_sbuf_tensor(f"gbuf{b}", [128, NCH2 // 128, H], FP32)
            for b in range(NBUF2)]
    lsem = nc.alloc_semaphore("lsem")
    gsems = [nc.alloc_semaphore(f"gsem{b}") for b in range(NBUF2)]
    ssems = [nc.alloc_semaphore(f"ssem{b}") for b in range(NBUF2)]

    with tile.TileContext(nc) as tc:
        with (
            tc.tile_pool(name="meta", bufs=1) as meta,
            tc.tile_pool(name="cpool", bufs=comb_bufs) as cpool,
        ):
            with tc.tile_critical():
                g = nc.gpsimd
                g.dma_start(out=gidx_sb[:], in_=gidx[:]).then_inc(lsem, 16)
                g.dma_start(out=sidx_sb[:], in_=sidx[:]).then_inc(lsem, 16)
                g.wait_ge(lsem, 32)

                def emit_gather(j):
                    g.dma_gather(
                        out_ap=bufs[j % NBUF2][:],
                        in_ap=x[:],
                        idxs_ap=gidx_sb[:, j * NCOLS2:(j + 1) * NCOLS2],
                        num_idxs=NCH2,
                        num_idxs_reg=NCH2,
                        elem_size=H,
                        queue_num=1,
                    ).then_inc(gsems[j % NBUF2], 16)

                # Issue-lookahead of 2 chunks over 4 rotating buffers: the
                # buffer-reuse wait for gather j+2 targets scatters of chunk
                # j-2, which drained ~2 chunk-periods ago - so neither SWDGE
                # ring ever runs dry.
                for j in range(min(2, n_chunks)):
                    emit_gather(j)
                col = 0
                # per-buffer cumulative scatter-op counts (for parity sems)
                scum = [0] * NBUF2
                cum_at = {}     # chunk j -> scum[j % NBUF2] after its scatters
                for j in range(n_chunks):
                    b = j % NBUF2
                    g.wait_ge(gsems[b], 16 * (j // NBUF2 + 1))
                    for c in range(NCH2 // 128):
                        for _ in range(passes[j][c]):
                            inst = g.indirect_dma_start(
                                out=disp[:],
                                out_offset=bass.IndirectOffsetOnAxis(
                                    ap=sidx_sb[:, col:col + 1], axis=0),
                                in_=bufs[b][:, c, :],
                                in_offset=None,
                                bounds_check=RPC - 1,
                                oob_is_err=False,
                            )
                            inst.then_inc(ssems[b], 16)
                            col += 1
                            scum[b] += 1
                    cum_at[j] = scum[b]
                    nxt = j + 2
                    if nxt < n_chunks:
                        prev_user = nxt - NBUF2   # chunk that last used this buf
                        if prev_user >= 0:
                            g.wait_ge(ssems[prev_user % NBUF2],
                                      16 * cum_at[prev_user])
                        emit_gather(nxt)
                for b in range(NBUF2):
                    if scum[b]:
                        g.wait_ge(ssems[b], 16 * scum[b])
            _emit_combine(nc, cpool, meta, w, xc, comb)
    nc.compile()
    return nc


def make_in_maps_dedup2(x, topk_idx, topk_weights):
    """dedup2 inputs. Returns (in_maps, tpe, n_chunks, passes)."""
    x = np.ascontiguousarray(x, dtype=np.float32)
    src_tok, tokens_per_expert = _routing(topk_idx)

    per_core = []
    for c in range(NCORES):
        sl = src_tok[c * RPC:(c + 1) * RPC]
        order = np.argsort(sl, kind="stable")
        u, starts, counts = np.unique(sl[order], return_index=True,
                                      return_counts=True)
        o2 = np.lexsort((u, -counts))   # count desc, token asc
        per_core.append((u[o2], starts[o2], counts[o2], order))

    n_chunks = max(int(np.ceil(len(u) / NCH2)) for u, _, _, _ in per_core)
    u_pad = n_chunks * NCH2
    passes = []
    for j in range(n_chunks):
        pj = []
        for c128 in range(NCH2 // 128):
            pos = j * NCH2 + c128 * 128
            p = 0
            for _, _, counts, _ in per_core:
                if pos < len(counts):
                    p = max(p, int(counts[pos]))
            pj.append(p)
        passes.append(tuple(pj))
    passes = tuple(passes)
    n_sc = sum(sum(p) for p in passes)

    in_maps = []
    for c in range(NCORES):
        u2, st2, cn2, order = per_core[c]
        nu = len(u2)
        up = np.full(u_pad, u2[0], dtype=np.int64)
        up[:nu] = u2
        cp = np.zeros(u_pad, dtype=np.int64)
        cp[:nu] = cn2
        sp_ = np.zeros(u_pad, dtype=np.int64)
        sp_[:nu] = st2

        gidx = np.zeros((128, n_chunks * NCOLS2), np.int16)
        sidx = np.full((128, n_sc), SENT, np.int32)
        col = 0
        for j in range(n_chunks):
            blk = up[j * NCH2:(j + 1) * NCH2].astype(np.int16)
            wrapped = blk.reshape(NCOLS2, 16).T
            for kk in range(8):
                gidx[16 * kk:16 * (kk + 1),
                     j * NCOLS2:(j + 1) * NCOLS2] = wrapped
            for c128 in range(NCH2 // 128):
                base = j * NCH2 + c128 * 128
                cj = cp[base:base + 128]
                stj = sp_[base:base + 128]
                for r in range(passes[j][c128]):
                    sel = np.nonzero(cj > r)[0]
                    if len(sel):
                        sidx[sel, col] = order[stj[sel] + r]
                    col += 1
        in_maps.append({"x": x, "gidx": gidx, "sidx": sidx,
                        **_common_inputs(x, topk_weights, c)})
    return in_maps, tokens_per_expert, n_chunks, passes


def _routing(topk_idx):
    flat = np.ascontiguousarray(topk_idx, dtype=np.int32).reshape(-1)
    sort_idx = np.argsort(flat, kind="stable")
    src_tok = (sort_idx // K).astype(np.int32)
    tokens_per_expert = np.bincount(flat, minlength=E).astype(topk_idx.dtype)
    return src_tok, tokens_per_expert


def _common_inputs(x, topk_weights, c):
    wc = np.ascontiguousarray(topk_weights, dtype=np.float32)[
        c * CTOK:(c + 1) * CTOK]
    w_arr = np.ascontiguousarray(
        wc.reshape(CTILES, 128, K).transpose(1, 0, 2).reshape(128, CTILES * K))
    return {"w": w_arr, "xc": np.ascontiguousarray(x[c * CTOK:(c + 1) * CTOK])}


def make_in_maps(x, topk_idx, topk_weights):
    """Simple variant inputs."""
    x = np.ascontiguousarray(x, dtype=np.float32)
    src_tok, tokens_per_expert = _routing(topk_idx)
    in_maps = []
    for c in range(NCORES):
        sl = src_tok[c * RPC:(c + 1) * RPC]
        idx_arr = np.ascontiguousarray(sl.reshape(NTILES, 128).T)
        in_maps.append({"x": x, "idx": idx_arr,
                        **_common_inputs(x, topk_weights, c)})
    return in_maps, tokens_per_expert


def make_in_maps_dedup(x, topk_idx, topk_weights):
    """Dedup variant inputs. Returns (in_maps, tpe, n_chunks, passes)."""
    x = np.ascontiguousarray(x, dtype=np.float32)
    src_tok, tokens_per_expert = _routing(topk_idx)

    per_core = []
    for c in range(NCORES):
        sl = src_tok[c * RPC:(c + 1) * RPC]
        order = np.argsort(sl, kind="stable")
        u, starts, counts = np.unique(sl[order], return_index=True,
                                      return_counts=True)
        per_core.append((u, order, starts, counts))

    n_chunks = max(int(np.ceil(len(u) / 128)) for u, _, _, _ in per_core)
    passes = []
    for j in range(n_chunks):
        pj = 0
        for u, _, _, counts in per_core:
            cj = counts[j * 128:(j + 1) * 128]
            if len(cj):
                pj = max(pj, int(cj.max()))
        passes.append(pj)
    passes = tuple(passes)
    sp = sum(passes)

    in_maps = []
    for c in range(NCORES):
        u, order, starts, counts = per_core[c]
        n_u = len(u)
        idxg = np.full((128, n_chunks), SENT, dtype=np.int32)
        idxs = np.full((128, sp), SENT, dtype=np.int32)
        col = 0
        for j in range(n_chunks):
            lo = j * 128
            nv = min(128, max(0, n_u - lo))
            if nv > 0:
                idxg[:nv, j] = u[lo:lo + nv]
            for r in range(passes[j]):
                if nv > 0:
                    crange = counts[lo:lo + nv]
                    sel = np.nonzero(crange > r)[0]
                    if len(sel):
                        idxs[sel, col] = order[starts[lo + sel] + r]
                col += 1
        in_maps.append({"x": x, "idxg": np.ascontiguousarray(idxg),
                        "idxs": np.ascontiguousarray(idxs),
                        **_common_inputs(x, topk_weights, c)})
    return in_maps, tokens_per_expert, n_chunks, passes


def make_in_maps_gsa(x, topk_idx, topk_weights):
    """gsa variant inputs. Returns (in_maps, tpe, n_chunks, passes)."""
    x = np.ascontiguousarray(x, dtype=np.float32)
    src_tok, tokens_per_expert = _routing(topk_idx)

    per_core = []
    for c in range(NCORES):
        sl = src_tok[c * RPC:(c + 1) * RPC]
        order = np.argsort(sl, kind="stable")
        u, starts, counts = np.unique(sl[order], return_index=True,
                                      return_counts=True)
        o2 = np.lexsort((u, -counts))   # count desc, token asc
        per_core.append((u[o2], starts[o2], counts[o2], order))

    n_chunks = max(int(np.ceil(len(u) / NCH)) for u, _, _, _ in per_core)
    u_pad = n_chunks * NCH
    passes = []
    for j in range(n_chunks):
        pj = 0
        for _, _, counts, _ in per_core:
            if j * NCH < len(counts):
                pj = max(pj, int(counts[j * NCH]))
        passes.append(pj)
    passes = tuple(passes)
    n_sc = sum(passes)

    in_maps = []
    for c in range(NCORES):
        u2, st2, cn2, order = per_core[c]
        nu = len(u2)
        up = np.full(u_pad, u2[0], dtype=np.int64)
        up[:nu] = u2
        cp = np.zeros(u_pad, dtype=np.int64)
        cp[:nu] = cn2
        sp_ = np.zeros(u_pad, dtype=np.int64)
        sp_[:nu] = st2

        gidx = np.zeros((128, n_chunks * NCOLS), np.int16)
        sidx = np.zeros((128, n_sc * NCOLS), np.int16)
        cnt = np.zeros((1, n_sc), np.int32)

        def put(arr, colblk, vals):
            # idx blocks are wrapped into 16 partitions and replicated for
            # each of the 8 GPSIMD Q7 cores (partition groups of 16)
            blk = vals.astype(np.int16).reshape(NCOLS, 16).T
            for kk in range(8):
                arr[16 * kk:16 * (kk + 1),
                    colblk * NCOLS:(colblk + 1) * NCOLS] = blk

        col = 0
        for j in range(n_chunks):
            put(gidx, j, up[j * NCH:(j + 1) * NCH])
            cj = cp[j * NCH:(j + 1) * NCH]
            stj = sp_[j * NCH:(j + 1) * NCH]
            assert (np.diff(cj) <= 0).all(), "counts must be descending"
            for r in range(passes[j]):
                m = int((cj > r).sum())
                dst = np.full(NCH, -1, np.int64)
                if m:
                    dst[:m] = order[stj[:m] + r]
                put(sidx, col, dst)
                cnt[0, col] = m
                col += 1
        in_maps.append({"x": x, "gidx": gidx, "sidx": sidx, "cnt": cnt,
                        **_common_inputs(x, topk_weights, c)})
    return in_maps, tokens_per_expert, n_chunks, passes


_module_cache = {}

STRATEGY = "dedup2"   # "simple" | "dedup" | "gsa" | "dedup2"


def kernel(x, topk_idx, topk_weights):
    if STRATEGY == "dedup2":
        in_maps, tpe, n_chunks, passes = make_in_maps_dedup2(
            x, topk_idx, topk_weights)
        key = ("dedup2", n_chunks, passes)
        if key not in _module_cache:
            _module_cache[key] = build_module_dedup2(n_chunks, passes)
    elif STRATEGY == "gsa":
        in_maps, tpe, n_chunks, passes = make_in_maps_gsa(
            x, topk_idx, topk_weights)
        key = ("gsa", n_chunks, passes)
        if key not in _module_cache:
            _module_cache[key] = build_module_gsa(n_chunks, passes)
    elif STRATEGY == "dedup":
        in_maps, tpe, n_chunks, passes = make_in_maps_dedup(
            x, topk_idx, topk_weights)
        key = ("dedup", n_chunks, passes)
        if key not in _module_cache:
            _module_cache[key] = build_module_dedup(n_chunks, passes)
    else:
        in_maps, tpe = make_in_maps(x, topk_idx, topk_weights)
        key = ("simple",)
        if key not in _module_cache:
            _module_cache[key] = build_module()
    nc = _module_cache[key]

    res = run_bass_kernel_spmd(nc, in_maps, core_ids=list(range(NCORES)))
    dispatched = np.concatenate([r["disp"] for r in res.results], axis=0)
    combined = np.concatenate([r["comb"] for r in res.results], axis=0)
    return combined, dispatched, tpe


# revision 16
# speedup vs baseline: 1.8426x; 1.0016x over previous
"""MoE all-to-all dispatch + combine (nn_EpAll2AllFusedOp) on 8 trn2 NeuronCores.

Semantics (matching the jax reference):
  flat_expert = topk_idx.reshape(T*K)
  sort_idx    = stable argsort of flat_expert
  dispatched  = x[sort_idx // K]                      # [T*K, H], expert-contiguous
  combined[t] = x[t] * sum_k topk_weights[t, k]      # gather-back of the K copies
  tokens_per_expert = histogram(flat_expert, 64)     # int32

Sharding: the dispatched buffer (T*K = 32768 rows, the 512 MB payload) is
split into 8 equal contiguous row slices - one per core (the expert-parallel
split of the sorted/A2A'd buffer, load-balanced by slot rather than raw
expert id). Each core produces its 4096-row slice of `dispatched` plus its
512-token slice of `combined`. The sort itself is O(T*K) integer metadata
computed on host; all tensor traffic runs on-device.

Two device strategies:
  simple: indirect-gather x[src_token[slot]] per 128-slot tile, direct store.
          ~151 MB/core of DMA traffic.
  dedup:  gather each DISTINCT source row once (~2700 of 4096 slots are
          unique), then indirect-scatter it to all its output slots,
          OOB-masked via bounds_check. ~128 MB/core -> ~15% less traffic.
"""

import numpy as np

import concourse.bass as bass
import concourse.mybir as mybir
import concourse.tile as tile
from concourse import bacc
from concourse.bass_utils import run_bass_kernel_spmd

T = 4096          # tokens
H = 4096          # hidden
K = 8             # topk
E = 64            # experts
NCORES = 8
RPC = T * K // NCORES       # dispatched rows per core = 4096
NTILES = RPC // 128         # gather tiles per core = 32
CTOK = T // NCORES          # combine tokens per core = 512
CTILES = CTOK // 128        # combine tiles per core = 4

FP32 = mybir.dt.float32
I32 = mybir.dt.int32
SENT = T                    # OOB sentinel row index (skipped via bounds_check;
                            # small value so index*row_stride never overflows i32)


def _emit_combine(nc, cpool, meta, w, xc, comb):
    w_sb = meta.tile([128, CTILES * K], FP32, name="w_sb")
    nc.sync.dma_start(out=w_sb[:], in_=w[:])
    ws = [meta.tile([128, 1], FP32, name=f"ws{j}", tag=f"ws{j}")
          for j in range(CTILES)]
    for j in range(CTILES):
        nc.vector.reduce_sum(
            out=ws[j][:], in_=w_sb[:, j * K:(j + 1) * K],
            axis=mybir.AxisListType.X,
        )
    for j in range(CTILES):
        xt = cpool.tile([128, H], FP32, name="xt", tag="c")
        nc.scalar.dma_start(out=xt[:], in_=xc[j * 128:(j + 1) * 128, :])
        nc.vector.tensor_scalar_mul(out=xt[:], in0=xt[:], scalar1=ws[j][:])
        nc.sync.dma_start(out=comb[j * 128:(j + 1) * 128, :], in_=xt[:])


def build_module(gather_bufs: int = 6, comb_bufs: int = 3):
    """Simple variant: one indirect gather + one direct store per 128 slots."""
    nc = bacc.Bacc("TRN2", num_devices=NCORES)
    x = nc.dram_tensor("x", [T, H], FP32, kind="ExternalInput")
    idx = nc.dram_tensor("idx", [128, NTILES], I32, kind="ExternalInput")
    w = nc.dram_tensor("w", [128, CTILES * K], FP32, kind="ExternalInput")
    xc = nc.dram_tensor("xc", [CTOK, H], FP32, kind="ExternalInput")
    disp = nc.dram_tensor("disp", [RPC, H], FP32, kind="ExternalOutput")
    comb = nc.dram_tensor("comb", [CTOK, H], FP32, kind="ExternalOutput")

    with tile.TileContext(nc) as tc:
        with (
            tc.tile_pool(name="meta", bufs=1) as meta,
            tc.tile_pool(name="gpool", bufs=gather_bufs) as gpool,
            tc.tile_pool(name="cpool", bufs=comb_bufs) as cpool,
        ):
            idx_sb = meta.tile([128, NTILES], I32, name="idx_sb")
            nc.sync.dma_start(out=idx_sb[:], in_=idx[:])
            for i in range(NTILES):
                g = gpool.tile([128, H], FP32, name="g", tag="g")
                nc.gpsimd.indirect_dma_start(
                    out=g[:],
                    out_offset=None,
                    in_=x[:],
                    in_offset=bass.IndirectOffsetOnAxis(
                        ap=idx_sb[:, i:i + 1], axis=0),
                )
                nc.sync.dma_start(out=disp[i * 128:(i + 1) * 128, :], in_=g[:])
            _emit_combine(nc, cpool, meta, w, xc, comb)
    nc.compile()
    return nc


def build_module_dedup(n_chunks: int, passes: tuple,
                       gather_bufs: int = 6, comb_bufs: int = 3):
    """Dedup variant: gather distinct rows once, indirect-scatter to all slots.

    n_chunks: number of 128-row unique-token chunks (same on all cores,
    OOB-padded). passes[j]: scatter passes for chunk j (max multiplicity over
    cores; masked slots move no bytes).
    """
    sp = sum(passes)
    nc = bacc.Bacc("TRN2", num_devices=NCORES)
    x = nc.dram_tensor("x", [T, H], FP32, kind="ExternalInput")
    idxg = nc.dram_tensor("idxg", [128, n_chunks], I32, kind="ExternalInput")
    idxs = nc.dram_tensor("idxs", [128, sp], I32, kind="ExternalInput")
    w = nc.dram_tensor("w", [128, CTILES * K], FP32, kind="ExternalInput")
    xc = nc.dram_tensor("xc", [CTOK, H], FP32, kind="ExternalInput")
    disp = nc.dram_tensor("disp", [RPC, H], FP32, kind="ExternalOutput")
    comb = nc.dram_tensor("comb", [CTOK, H], FP32, kind="ExternalOutput")

    with tile.TileContext(nc) as tc:
        with (
            tc.tile_pool(name="meta", bufs=1) as meta,
            tc.tile_pool(name="gpool", bufs=gather_bufs) as gpool,
            tc.tile_pool(name="cpool", bufs=comb_bufs) as cpool,
        ):
            idxg_sb = meta.tile([128, n_chunks], I32, name="idxg_sb")
            nc.sync.dma_start(out=idxg_sb[:], in_=idxg[:])
            idxs_sb = meta.tile([128, sp], I32, name="idxs_sb")
            nc.sync.dma_start(out=idxs_sb[:], in_=idxs[:])
            col = 0
            for j in range(n_chunks):
                g = gpool.tile([128, H], FP32, name="g", tag="g")
                nc.gpsimd.indirect_dma_start(
                    out=g[:],
                    out_offset=None,
                    in_=x[:],
                    in_offset=bass.IndirectOffsetOnAxis(
                        ap=idxg_sb[:, j:j + 1], axis=0),
                    bounds_check=T - 1,
                    oob_is_err=False,
                )
                for _ in range(passes[j]):
                    nc.gpsimd.indirect_dma_start(
                        out=disp[:],
                        out_offset=bass.IndirectOffsetOnAxis(
                            ap=idxs_sb[:, col:col + 1], axis=0),
                        in_=g[:],
                        in_offset=None,
                        bounds_check=RPC - 1,
                        oob_is_err=False,
                    )
                    col += 1
            _emit_combine(nc, cpool, meta, w, xc, comb)
    nc.compile()
    return nc


NCH = 512                   # unique-row chunk size for the gsa strategy
NCOLS = NCH // 16           # int16 index columns per chunk block


def build_module_gsa(n_chunks: int, passes: tuple, comb_bufs: int = 3,
                     scatter_queue: int = 1):
    """Gather/scatter-add variant.

    Each core gathers its distinct source rows once (dma_gather, NCH rows per
    op, ~46 MB instead of 67 MB) and fans each row out to all its output
    slots with dma_scatter_add into the zero-initialized disp buffer
    (CCE-add into zeros == write; every slot is written exactly once).
    The dispatch pipeline lives in a tile_critical section with manual
    semaphores - pure gpsimd, double-buffered, so Tile's conservative
    whole-tensor WAW tracking can't serialize the scatter stream. Per-core
    active-index counts are runtime values loaded into a gpsimd register
    (reg_load) so the SPMD program stays identical across cores.
    """
    n_sc = sum(passes)
    nc = bacc.Bacc("TRN2", num_devices=NCORES,
                   num_swdge_queues=max(2, scatter_queue + 1))
    x = nc.dram_tensor("x", [T, H], FP32, kind="ExternalInput")
    gidx = nc.dram_tensor("gidx", [128, n_chunks * NCOLS], mybir.dt.int16,
                          kind="ExternalInput")
    sidx = nc.dram_tensor("sidx", [128, n_sc * NCOLS], mybir.dt.int16,
                          kind="ExternalInput")
    cnt = nc.dram_tensor("cnt", [1, n_sc], I32, kind="ExternalInput")
    w = nc.dram_tensor("w", [128, CTILES * K], FP32, kind="ExternalInput")
    xc = nc.dram_tensor("xc", [CTOK, H], FP32, kind="ExternalInput")
    disp = nc.dram_tensor("disp", [RPC, H], FP32, kind="ExternalOutput")
    comb = nc.dram_tensor("comb", [CTOK, H], FP32, kind="ExternalOutput")

    gidx_sb = nc.alloc_sbuf_tensor("gidx_sb", [128, n_chunks * NCOLS],
                                   mybir.dt.int16)
    sidx_sb = nc.alloc_sbuf_tensor("sidx_sb", [128, n_sc * NCOLS],
                                   mybir.dt.int16)
    cnt_sb = nc.alloc_sbuf_tensor("cnt_sb", [1, n_sc], I32)
    bufs = [nc.alloc_sbuf_tensor(f"gbuf{b}", [128, NCH // 128, H], FP32)
            for b in range(2)]
    lsem = nc.alloc_semaphore("lsem")
    gsems = [nc.alloc_semaphore("gsem0"), nc.alloc_semaphore("gsem1")]
    ssem = nc.alloc_semaphore("ssem")

    with tile.TileContext(nc) as tc:
        with (
            tc.tile_pool(name="meta", bufs=1) as meta,
            tc.tile_pool(name="cpool", bufs=comb_bufs) as cpool,
        ):
            with tc.tile_critical():
                g = nc.gpsimd
                with g.register("rcnt") as rcnt:
                    g.dma_start(out=gidx_sb[:], in_=gidx[:]).then_inc(lsem, 16)
                    g.dma_start(out=sidx_sb[:], in_=sidx[:]).then_inc(lsem, 16)
                    g.dma_start(out=cnt_sb[:], in_=cnt[:]).then_inc(lsem, 16)
                    g.wait_ge(lsem, 48)

                    def emit_gather(j):
                        g.dma_gather(
                            out_ap=bufs[j % 2][:],
                            in_ap=x[:],
                            idxs_ap=gidx_sb[:, j * NCOLS:(j + 1) * NCOLS],
                            num_idxs=NCH,
                            num_idxs_reg=NCH,
                            elem_size=H,
                            queue_num=0,
                        ).then_inc(gsems[j % 2], 16)

                    for j in range(min(2, n_chunks)):
                        emit_gather(j)
                    col = 0
                    for j in range(n_chunks):
                        g.wait_ge(gsems[j % 2], 16 * (j // 2 + 1))
                        for _ in range(passes[j]):
                            g.reg_load(rcnt, cnt_sb[0:1, col:col + 1])
                            g.dma_scatter_add(
                                out_ap=disp[:],
                                in_ap=bufs[j % 2][:],
                                idxs_ap=sidx_sb[:, col * NCOLS:(col + 1) * NCOLS],
                                num_idxs=NCH,
                                num_idxs_reg=rcnt,
                                elem_size=H,
                                queue_num=scatter_queue,
                            ).then_inc(ssem, 16)
                            col += 1
                        if j + 2 < n_chunks:
                            g.wait_ge(ssem, 16 * col)
                            emit_gather(j + 2)
                    g.wait_ge(ssem, 16 * col)
            _emit_combine(nc, cpool, meta, w, xc, comb)
    nc.compile()
    return nc


NCH2 = 256                  # unique-row chunk size for dedup2
NCOLS2 = NCH2 // 16
NBUF2 = 4                   # gather buffers (pipeline depth)


def build_module_dedup2(n_chunks: int, passes: tuple, comb_bufs: int = 3):
    """Dedup v2: coalescing dma_gather of unique rows (queue 0) + plain
    indirect-DMA scatter of each row to its output slots (bypass write,
    OOB-masked via bounds_check, queue 1). Manual-semaphore pipeline inside
    a gpsimd-only critical section so Tile's whole-tensor WAW tracking can't
    serialize the scatter stream. passes[j] is a tuple of per-128-column
    scatter pass counts for chunk j (max over cores).
    """
    n_sc = sum(sum(p) for p in passes)
    nc = bacc.Bacc("TRN2", num_devices=NCORES, num_swdge_queues=2)
    x = nc.dram_tensor("x", [T, H], FP32, kind="ExternalInput")
    gidx = nc.dram_tensor("gidx", [128, n_chunks * NCOLS2], mybir.dt.int16,
                          kind="ExternalInput")
    sidx = nc.dram_tensor("sidx", [128, n_sc], I32, kind="ExternalInput")
    w = nc.dram_tensor("w", [128, CTILES * K], FP32, kind="ExternalInput")
    xc = nc.dram_tensor("xc", [CTOK, H], FP32, kind="ExternalInput")
    disp = nc.dram_tensor("disp", [RPC, H], FP32, kind="ExternalOutput")
    comb = nc.dram_tensor("comb", [CTOK, H], FP32, kind="ExternalOutput")

    gidx_sb = nc.alloc_sbuf_tensor("gidx_sb", [128, n_chunks * NCOLS2],
                                   mybir.dt.int16)
    sidx_sb = nc.alloc_sbuf_tensor("sidx_sb", [128, n_sc], I32)
    bufs = [nc.alloc_sbuf_tensor(f"gbuf{b}", [128, NCH2 // 128, H], FP32)
            for b in range(NBUF2)]
    lsem = nc.alloc_semaphore("lsem")
    gsems = [nc.alloc_semaphore(f"gsem{b}") for b in range(NBUF2)]
    ssems = [nc.alloc_semaphore(f"ssem{b}") for b in range(NBUF2)]

    with tile.TileContext(nc) as tc:
        with (
            tc.tile_pool(name="meta", bufs=1) as meta,
            tc.tile_pool(name="cpool", bufs=comb_bufs) as cpool,
        ):
            with tc.tile_critical():
                g = nc.gpsimd
                g.dma_start(out=gidx_sb[:], in_=gidx[:]).then_inc(lsem, 16)
                g.dma_start(out=sidx_sb[:], in_=sidx[:]).then_inc(lsem, 16)

                def emit_gather(j):
                    return g.dma_gather(
                        out_ap=bufs[j % NBUF2][:],
                        in_ap=x[:],
                        idxs_ap=gidx_sb[:, j * NCOLS2:(j + 1) * NCOLS2],
                        num_idxs=NCH2,
                        num_idxs_reg=NCH2,
                        elem_size=H,
                        queue_num=1,
                    ).then_inc(gsems[j % NBUF2], 16)

                g.wait_ge(lsem, 32)
                # Issue-lookahead of 2 chunks over 4 rotating buffers: the
                # buffer-reuse wait for gather j+2 targets scatters of chunk
                # j-2, which drained ~2 chunk-periods ago - so neither SWDGE
                # ring ever runs dry.
                for j in range(min(2, n_chunks)):
                    emit_gather(j)
                col = 0
                # per-buffer cumulative scatter-op counts (for parity sems)
                scum = [0] * NBUF2
                cum_at = {}     # chunk j -> scum[j % NBUF2] after its scatters
                for j in range(n_chunks):
                    b = j % NBUF2
                    g.wait_ge(gsems[b], 16 * (j // NBUF2 + 1))
                    for c in range(NCH2 // 128):
                        for _ in range(passes[j][c]):
                            inst = g.indirect_dma_start(
                                out=disp[:],
                                out_offset=bass.IndirectOffsetOnAxis(
                                    ap=sidx_sb[:, col:col + 1], axis=0),
                                in_=bufs[b][:, c, :],
                                in_offset=None,
                                bounds_check=RPC - 1,
                                oob_is_err=False,
                            )
                            inst.then_inc(ssems[b], 16)
                            col += 1
                            scum[b] += 1
                    cum_at[j] = scum[b]
                    nxt = j + 2
                    if nxt < n_chunks:
                        prev_user = nxt - NBUF2   # chunk that last used this buf
                        if prev_user >= 0:
                            g.wait_ge(ssems[prev_user % NBUF2],
                                      16 * cum_at[prev_user])
                        emit_gather(nxt)
                for b in range(NBUF2):
                    if scum[b]:
                        g.wait_ge(ssems[b], 16 * scum[b])
            _emit_combine(nc, cpool, meta, w, xc, comb)
    nc.compile()
    return nc


def make_in_maps_dedup2(x, topk_idx, topk_weights):
    """dedup2 inputs. Returns (in_maps, tpe, n_chunks, passes)."""
    x = np.ascontiguousarray(x, dtype=np.float32)
    src_tok, tokens_per_expert = _routing(topk_idx)

    per_core = []
    for c in range(NCORES):
        sl = src_tok[c * RPC:(c + 1) * RPC]
        order = np.argsort(sl, kind="stable")
        u, starts, counts = np.unique(sl[order], return_index=True,
                                      return_counts=True)
        o2 = np.lexsort((u, -counts))   # count desc, token asc
        per_core.append((u[o2], starts[o2], counts[o2], order))

    n_chunks = max(int(np.ceil(len(u) / NCH2)) for u, _, _, _ in per_core)
    u_pad = n_chunks * NCH2
    passes = []
    for j in range(n_chunks):
        pj = []
        for c128 in range(NCH2 // 128):
            pos = j * NCH2 + c128 * 128
            p = 0
            for _, _, counts, _ in per_core:
                if pos < len(counts):
                    p = max(p, int(counts[pos]))
            pj.append(p)
        passes.append(tuple(pj))
    passes = tuple(passes)
    n_sc = sum(sum(p) for p in passes)

    in_maps = []
    for c in range(NCORES):
        u2, st2, cn2, order = per_core[c]
        nu = len(u2)
        up = np.full(u_pad, u2[0], dtype=np.int64)
        up[:nu] = u2
        cp = np.zeros(u_pad, dtype=np.int64)
        cp[:nu] = cn2
        sp_ = np.zeros(u_pad, dtype=np.int64)
        sp_[:nu] = st2

        gidx = np.zeros((128, n_chunks * NCOLS2), np.int16)
        sidx = np.full((128, n_sc), SENT, np.int32)
        col = 0
        for j in range(n_chunks):
            blk = up[j * NCH2:(j + 1) * NCH2].astype(np.int16)
            wrapped = blk.reshape(NCOLS2, 16).T
            for kk in range(8):
                gidx[16 * kk:16 * (kk + 1),
                     j * NCOLS2:(j + 1) * NCOLS2] = wrapped
            for c128 in range(NCH2 // 128):
                base = j * NCH2 + c128 * 128
                cj = cp[base:base + 128]
                stj = sp_[base:base + 128]
                for r in range(passes[j][c128]):
                    sel = np.nonzero(cj > r)[0]
                    if len(sel):
                        sidx[sel, col] = order[stj[sel] + r]
                    col += 1
        in_maps.append({"x": x, "gidx": gidx, "sidx": sidx,
                        **_common_inputs(x, topk_weights, c)})
    return in_maps, tokens_per_expert, n_chunks, passes


def _routing(topk_idx):
    flat = np.ascontiguousarray(topk_idx, dtype=np.int32).reshape(-1)
    sort_idx = np.argsort(flat, kind="stable")
    src_tok = (sort_idx // K).astype(np.int32)
    tokens_per_expert = np.bincount(flat, minlength=E).astype(topk_idx.dtype)
    return src_tok, tokens_per_expert


def _common_inputs(x, topk_weights, c):
    wc = np.ascontiguousarray(topk_weights, dtype=np.float32)[
        c * CTOK:(c + 1) * CTOK]
    w_arr = np.ascontiguousarray(
        wc.reshape(CTILES, 128, K).transpose(1, 0, 2).reshape(128, CTILES * K))
    return {"w": w_arr, "xc": np.ascontiguousarray(x[c * CTOK:(c + 1) * CTOK])}


def make_in_maps(x, topk_idx, topk_weights):
    """Simple variant inputs."""
    x = np.ascontiguousarray(x, dtype=np.float32)
    src_tok, tokens_per_expert = _routing(topk_idx)
    in_maps = []
    for c in range(NCORES):
        sl = src_tok[c * RPC:(c + 1) * RPC]
        idx_arr = np.ascontiguousarray(sl.reshape(NTILES, 128).T)
        in_maps.append({"x": x, "idx": idx_arr,
                        **_common_inputs(x, topk_weights, c)})
    return in_maps, tokens_per_expert


def make_in_maps_dedup(x, topk_idx, topk_weights):
    """Dedup variant inputs. Returns (in_maps, tpe, n_chunks, passes)."""
    x = np.ascontiguousarray(x, dtype=np.float32)
    src_tok, tokens_per_expert = _routing(topk_idx)

    per_core = []
    for c in range(NCORES):
        sl = src_tok[c * RPC:(c + 1) * RPC]
        order = np.argsort(sl, kind="stable")
        u, starts, counts = np.unique(sl[order], return_index=True,
                                      return_counts=True)
        per_core.append((u, order, starts, counts))

    n_chunks = max(int(np.ceil(len(u) / 128)) for u, _, _, _ in per_core)
    passes = []
    for j in range(n_chunks):
        pj = 0
        for u, _, _, counts in per_core:
            cj = counts[j * 128:(j + 1) * 128]
            if len(cj):
                pj = max(pj, int(cj.max()))
        passes.append(pj)
    passes = tuple(passes)
    sp = sum(passes)

    in_maps = []
    for c in range(NCORES):
        u, order, starts, counts = per_core[c]
        n_u = len(u)
        idxg = np.full((128, n_chunks), SENT, dtype=np.int32)
        idxs = np.full((128, sp), SENT, dtype=np.int32)
        col = 0
        for j in range(n_chunks):
            lo = j * 128
            nv = min(128, max(0, n_u - lo))
            if nv > 0:
                idxg[:nv, j] = u[lo:lo + nv]
            for r in range(passes[j]):
                if nv > 0:
                    crange = counts[lo:lo + nv]
                    sel = np.nonzero(crange > r)[0]
                    if len(sel):
                        idxs[sel, col] = order[starts[lo + sel] + r]
                col += 1
        in_maps.append({"x": x, "idxg": np.ascontiguousarray(idxg),
                        "idxs": np.ascontiguousarray(idxs),
                        **_common_inputs(x, topk_weights, c)})
    return in_maps, tokens_per_expert, n_chunks, passes


def make_in_maps_gsa(x, topk_idx, topk_weights):
    """gsa variant inputs. Returns (in_maps, tpe, n_chunks, passes)."""
    x = np.ascontiguousarray(x, dtype=np.float32)
    src_tok, tokens_per_expert = _routing(topk_idx)

    per_core = []
    for c in range(NCORES):
        sl = src_tok[c * RPC:(c + 1) * RPC]
        order = np.argsort(sl, kind="stable")
        u, starts, counts = np.unique(sl[order], return_index=True,
                                      return_counts=True)
        o2 = np.lexsort((u, -counts))   # count desc, token asc
        per_core.append((u[o2], starts[o2], counts[o2], order))

    n_chunks = max(int(np.ceil(len(u) / NCH)) for u, _, _, _ in per_core)
    u_pad = n_chunks * NCH
    passes = []
    for j in range(n_chunks):
        pj = 0
        for _, _, counts, _ in per_core:
            if j * NCH < len(counts):
                pj = max(pj, int(counts[j * NCH]))
        passes.append(pj)
    passes = tuple(passes)
    n_sc = sum(passes)

    in_maps = []
    for c in range(NCORES):
        u2, st2, cn2, order = per_core[c]
        nu = len(u2)
        up = np.full(u_pad, u2[0], dtype=np.int64)
        up[:nu] = u2
        cp = np.zeros(u_pad, dtype=np.int64)
        cp[:nu] = cn2
        sp_ = np.zeros(u_pad, dtype=np.int64)
        sp_[:nu] = st2

        gidx = np.zeros((128, n_chunks * NCOLS), np.int16)
        sidx = np.zeros((128, n_sc * NCOLS), np.int16)
        cnt = np.zeros((1, n_sc), np.int32)

        def put(arr, colblk, vals):
            # idx blocks are wrapped into 16 partitions and replicated for
            # each of the 8 GPSIMD Q7 cores (partition groups of 16)
            blk = vals.astype(np.int16).reshape(NCOLS, 16).T
            for kk in range(8):
                arr[16 * kk:16 * (kk + 1),
                    colblk * NCOLS:(colblk + 1) * NCOLS] = blk

        col = 0
        for j in range(n_chunks):
            put(gidx, j, up[j * NCH:(j + 1) * NCH])
            cj = cp[j * NCH:(j + 1) * NCH]
            stj = sp_[j * NCH:(j + 1) * NCH]
            assert (np.diff(cj) <= 0).all(), "counts must be descending"
            for r in range(passes[j]):
                m = int((cj > r).sum())
                dst = np.full(NCH, -1, np.int64)
                if m:
                    dst[:m] = order[stj[:m] + r]
                put(sidx, col, dst)
                cnt[0, col] = m
                col += 1
        in_maps.append({"x": x, "gidx": gidx, "sidx": sidx, "cnt": cnt,
                        **_common_inputs(x, topk_weights, c)})
    return in_maps, tokens_per_expert, n_chunks, passes


_module_cache = {}

STRATEGY = "dedup2"   # "simple" | "dedup" | "gsa" | "dedup2"


def kernel(x, topk_idx, topk_weights):
    if STRATEGY == "dedup2":
        in_maps, tpe, n_chunks, passes = make_in_maps_dedup2(
            x, topk_idx, topk_weights)
        key = ("dedup2", n_chunks, passes)
        if key not in _module_cache:
            _module_cache[key] = build_module_dedup2(n_chunks, passes)
    elif STRATEGY == "gsa":
        in_maps, tpe, n_chunks, passes = make_in_maps_gsa(
            x, topk_idx, topk_weights)
        key = ("gsa", n_chunks, passes)
        if key not in _module_cache:
            _module_cache[key] = build_module_gsa(n_chunks, passes)
    elif STRATEGY == "dedup":
        in_maps, tpe, n_chunks, passes = make_in_maps_dedup(
            x, topk_idx, topk_weights)
        key = ("dedup", n_chunks, passes)
        if key not in _module_cache:
            _module_cache[key] = build_module_dedup(n_chunks, passes)
    else:
        in_maps, tpe = make_in_maps(x, topk_idx, topk_weights)
        key = ("simple",)
        if key not in _module_cache:
            _module_cache[key] = build_module()
    nc = _module_cache[key]

    res = run_bass_kernel_spmd(nc, in_maps, core_ids=list(range(NCORES)))
    dispatched = np.concatenate([r["disp"] for r in res.results], axis=0)
    combined = np.concatenate([r["comb"] for r in res.results], axis=0)
    return combined, dispatched, tpe
